# revision 1
# baseline (speedup 1.0000x reference)
"""Trainium2 Bass kernel for a BinaryNet conv block.

Pipeline (per core, data-parallel over batch):
  sign(x) -> conv3x3(sign(w1)) -> BN1 -> sign -> conv3x3(sign(w2))
          -> maxpool2x2 -> BN2

Implementation notes:
  - Activations are +-0.5, weights +-1.0 in fp8e4 (exactly representable);
    convs run as 9 shifted-window matmuls with DoubleRow perf mode (K=256
    contraction per instruction), accumulating exactly into fp32 PSUM.
  - BN1+sign is fused into one ScalarE Sign activation against a
    host-precomputed per-channel threshold. Conv outputs are exact
    integers, so an integer cutoff k_c reproduces the reference's fp32
    sign decisions bit-exactly.
  - Spatial layout is channel-major [ci, y*(W+2)+x] with a zero border so
    the 9 taps are just constant AP offsets.
  - The bass2jax/pseudo-DMA path allows only ONE sync wait per DMA and has
    8 DMA lanes, so the kernel uses exactly 8 DMAs (1 packed consts, 4 x
    loads into DISTINCT tiles, 3 y stores); no DMA destination tile is
    ever reused, so every DMA needs at most one semaphore wait.
  - Emission is software-pipelined (input prep leads convs by one image)
    and the pool/BN2/output-transpose tail is emitted per conv2 stretch,
    which keeps the PE gapless between images.
"""

import os
import numpy as np

os.environ.setdefault("MYCRO_LOCAL_CACHE", "1")

N_CORES = 8
C = 256
NCHUNK = 2  # channel chunks of 128
KP = 128

# packed consts layout (bytes per partition)
W1_OFF = 0
W2_OFF = 4608
NT1_OFF = 9216  # f32 [2]
S2_OFF = 9224
B2_OFF = 9232
CONST_B = 9248


def build_program(B, H, W, psum_stretch=1024, conv_bufs=3):
    """Build the per-core Bass program. B images of HxWxC per core."""
    import concourse.bass as bass
    import concourse.bacc as bacc
    import concourse.tile as tile
    from concourse import mybir

    F32 = mybir.dt.float32
    FP8 = mybir.dt.float8e4
    BF16 = mybir.dt.bfloat16
    U8 = mybir.dt.uint8
    DR = mybir.MatmulPerfMode.DoubleRow
    Alu = mybir.AluOpType
    Act = mybir.ActivationFunctionType

    Hp, Wp = H + 2, W + 2
    S_pad = Hp * Wp
    DOFF = 32  # left zero pad inside each channel-chunk row buffer
    S_chunk = ((S_pad + DOFF + 32 + 15) // 16) * 16  # right pad >= 32
    RB = 2 * W  # transpose block = 2 image rows
    assert RB <= 128
    NB = H // 2  # transpose blocks per image
    G = 7 if NB % 7 == 0 else (2 if NB % 2 == 0 else 1)  # blocks per psum group
    NG = NB // G
    PO = (H // 2) * (W // 2)
    OB = min(112, PO)  # output transpose block (partitions)
    assert PO % OB == 0
    NOB = PO // OB

    def split_stretch(total, step):
        out, a = [], 0
        while a < total:
            out.append((a, min(step, total - a)))
            a += step
        return out

    max_rows = (psum_stretch // Wp) // 2 * 2
    row_groups = []
    r = 0
    while r < H:
        g = min(max_rows, H - r)
        row_groups.append((r, g))
        r += g
    conv2_st = [((1 + r0) * Wp, rg * Wp, r0, rg) for r0, rg in row_groups]
    conv1_st = conv2_st
    PS_COLS = psum_stretch

    nc = bacc.Bacc("TRN2", target_bir_lowering=False, debug=False)

    x_h = nc.dram_tensor("x", [B, H * W, C], F32, kind="ExternalInput")
    cb_h = nc.dram_tensor("cb", [KP, CONST_B], U8, kind="ExternalInput")
    y_h = nc.dram_tensor("y", [B, PO, C], F32, kind="ExternalOutput")

    def dram_ap(handle, offset, dims):
        return bass.AP(
            tensor=handle.ap().tensor, offset=offset, ap=[list(d) for d in dims]
        )

    with tile.TileContext(nc) as tc:
        from contextlib import ExitStack

        with ExitStack() as ctx:
            consts = ctx.enter_context(tc.tile_pool(name="consts", bufs=1))
            xnat_p = ctx.enter_context(tc.tile_pool(name="xnat", bufs=1))
            xsg_p = ctx.enter_context(tc.tile_pool(name="xsg", bufs=2))
            xsT_p = ctx.enter_context(tc.tile_pool(name="xsT", bufs=2))
            hsT_p = ctx.enter_context(tc.tile_pool(name="hsT", bufs=2))
            pr_p = ctx.enter_context(tc.tile_pool(name="prp", bufs=2))
            po_p = ctx.enter_context(tc.tile_pool(name="pop", bufs=2))
            onat_p = ctx.enter_context(tc.tile_pool(name="onat", bufs=1))
            convp = ctx.enter_context(tc.tile_pool(name="convp", bufs=conv_bufs, space="PSUM"))
            tp_p = ctx.enter_context(tc.tile_pool(name="tpp", bufs=2, space="PSUM"))

            # --- packed constants: one DMA (issued after img0's x load so
            # the input pipeline wins the DMA bandwidth race), bitcast views
            cb = consts.tile([KP, CONST_B], U8)
            cb_dma = [False]

            def load_consts():
                if not cb_dma[0]:
                    nc.sync.dma_start(out=cb, in_=cb_h.ap())
                    cb_dma[0] = True
            w1sb = cb[:, W1_OFF : W1_OFF + 4608].bitcast(FP8).rearrange(
                "p (t j k m) -> p t j k m", t=9, j=NCHUNK, k=2
            )
            w2sb = cb[:, W2_OFF : W2_OFF + 4608].bitcast(FP8).rearrange(
                "p (t j k m) -> p t j k m", t=9, j=NCHUNK, k=2
            )
            # identities built on-device (GPSIMD) so transposes don't wait
            # for the big consts DMA
            from concourse import masks

            id8sb = consts.tile([KP, KP], BF16)
            id32sb = consts.tile([KP, KP], F32)
            masks.make_identity(nc, id8sb)
            masks.make_identity(nc, id32sb)
            nt1sb = cb[:, NT1_OFF : NT1_OFF + 8].bitcast(F32)
            s2sb = cb[:, S2_OFF : S2_OFF + 8].bitcast(F32)
            b2sb = cb[:, B2_OFF : B2_OFF + 8].bitcast(F32)

            def border_memsets(buf):
                # rows 0 and H+1, left/right pads, and border cols {0, W+1} of
                # rows 1..H. Interior writes never touch these bytes, so all
                # zeroing happens up front with no WAW serialization.
                nc.vector.memset(buf[:, :, 0 : DOFF + Wp], 0.0)
                nc.vector.memset(buf[:, :, DOFF + (H + 1) * Wp : S_chunk], 0.0)
                rows = buf[:, :, DOFF + Wp : DOFF + (H + 1) * Wp].rearrange(
                    "p j (r w) -> p j r w", w=Wp
                )
                nc.vector.memset(rows[:, :, :, 0 :: (W + 1)], 0.0)

            def conv(inbuf, wsb, stretches, psum_tiles_cb):
                for si, st in enumerate(stretches):
                    cs, cn = st[0], st[1]
                    for j in range(NCHUNK):
                        ps = convp.tile([KP, PS_COLS], F32, tag="cv", name=f"cv{si}{j}")
                        for t in range(9):
                            dy, dx = t // 3, t % 3
                            off = (dy - 1) * Wp + (dx - 1)
                            lhsT = wsb[:, t, j]
                            for c0 in range(0, cn, 512):
                                n = min(512, cn - c0)
                                a = DOFF + cs + off + c0
                                rhs = inbuf[:, :, a : a + n]
                                nc.tensor.matmul(
                                    ps[:, c0 : c0 + n],
                                    lhsT,
                                    rhs,
                                    start=(t == 0),
                                    stop=(t == 8),
                                    perf_mode=DR,
                                )
                        psum_tiles_cb(si, j, ps, st)

            # output DMA groups: {0,1,2}, {3 in two pieces} for B=4
            if B == 4:
                out_groups = [(0, 3), (3, 1)]
            else:
                out_groups = [(i, 1) for i in range(B)]
            SPLIT_LAST = B == 4 and NOB >= 2
            grp_of = {}
            for g0, gn in out_groups:
                for i in range(g0, g0 + gn):
                    grp_of[i] = (g0, gn)
            onat_box = [None]
            xsT_tiles = {}
            xn_views = {}

            def get_xn(img):
                if img in xn_views:
                    return xn_views.pop(img)
                if B == 4 and img == 0:
                    xn = xnat_p.tile([RB, NB, C], F32, tag="xn0", name="xn0")
                    h1 = NB // 2
                    nc.sync.dma_start(
                        out=xn[:, :h1, :],
                        in_=dram_ap(x_h, 0, [[C, RB], [RB * C, h1], [1, C]]),
                    )
                    nc.sync.dma_start(
                        out=xn[:, h1:, :],
                        in_=dram_ap(
                            x_h, h1 * RB * C, [[C, RB], [RB * C, NB - h1], [1, C]]
                        ),
                    )
                    return xn
                if B == 4 and img == 2:
                    # one DMA covering images 2 and 3 (contiguous in DRAM)
                    xn2 = xnat_p.tile([RB, 2 * NB, C], F32, tag="xn23", name="xn23")
                    nc.sync.dma_start(
                        out=xn2,
                        in_=dram_ap(
                            x_h, 2 * H * W * C, [[C, RB], [RB * C, 2 * NB], [1, C]]
                        ),
                    )
                    xn_views[3] = xn2[:, NB:, :]
                    return xn2[:, :NB, :]
                xn = xnat_p.tile([RB, NB, C], F32, tag=f"xn{img}", name=f"xn{img}")
                nc.sync.dma_start(
                    out=xn,
                    in_=dram_ap(
                        x_h, img * H * W * C, [[C, RB], [RB * C, NB], [1, C]]
                    ),
                )
                return xn

            def prep_input(img):
                # one DMA + sign + PE transpose into channel-major fp8 layout
                xn = get_xn(img)
                xsT = xsT_p.tile(
                    [KP, NCHUNK, S_chunk], FP8, tag="xsT", name=f"xsT{img}"
                )
                border_memsets(xsT)
                for g in range(NG):
                    xg = xsg_p.tile([RB, G, C], BF16, tag="xg", name=f"xg{img}{g}")
                    nc.vector.tensor_scalar(
                        xg, xn[:, g * G : (g + 1) * G, :], 0.0, 0.5,
                        Alu.is_ge, Alu.subtract,
                    )
                    for j in range(NCHUNK):
                        tp = tp_p.tile(
                            [KP, G, RB], BF16, tag="tp", name=f"tpi{img}{g}{j}"
                        )
                        for b in range(G):
                            nc.tensor.transpose(
                                tp[:, b, :],
                                xg[:, b, j * KP : (j + 1) * KP],
                                id8sb[:RB, :RB],
                            )
                        srcv = tp[:, :, :].rearrange("p g (r w) -> p (g r) w", w=W)
                        a0 = DOFF + (1 + 2 * G * g) * Wp
                        dst = xsT[:, j, a0 : a0 + 2 * G * Wp].rearrange(
                            "p (r w) -> p r w", w=Wp
                        )[:, :, 1 : 1 + W]
                        # split the scatter copies across ACT/DVE so the
                        # sign->copy chain doesn't serialize on one engine
                        if j == 0:
                            nc.scalar.copy(dst, srcv)
                        else:
                            nc.vector.tensor_copy(dst, srcv)
                xsT_tiles[img] = xsT

            def run_convs(img):
                xsT = xsT_tiles.pop(img)
                g0, gn = grp_of[img]
                # ---------- conv1 -> BN1+sign ----------
                hsT = hsT_p.tile(
                    [KP, NCHUNK, S_chunk], FP8, tag="hsT", name=f"hsT{img}"
                )
                border_memsets(hsT)

                def bnsign(si, j, ps, st):
                    cs, cn, r0, rg = st
                    dstv = hsT[:, j, DOFF + cs : DOFF + cs + cn].rearrange(
                        "p (r w) -> p r w", w=Wp
                    )[:, :, 1 : 1 + W]
                    srcv = ps[:, :cn].rearrange("p (r w) -> p r w", w=Wp)[
                        :, :, 1 : 1 + W
                    ]
                    nc.scalar.activation(
                        dstv, srcv, Act.Sign, bias=nt1sb[:, j : j + 1], scale=1.0
                    )

                conv(xsT, w1sb, conv1_st, bnsign)

                # ---------- conv2 -> pool -> BN2 -> transpose (per stretch) ---
                if img == g0:
                    onat_box[0] = onat_p.tile(
                        [OB, max(gn, 1), NOB, C], F32, tag="on", name=f"on{img}"
                    )
                onat = onat_box[0]
                pr_tiles = [
                    pr_p.tile([KP, H // 2, W], F32, tag="pr", name=f"pr{img}{j}")
                    for j in range(NCHUNK)
                ]
                pooled_tiles = [
                    po_p.tile([KP, PO], F32, tag="pooled", name=f"pl{img}{j}")
                    for j in range(NCHUNK)
                ]
                max_pairs = max(rg for _, rg in row_groups) // 2
                WH = W // 2

                def pool1(si, j, ps, st):
                    cs, cn, r0, rg = st
                    rows = ps[:, : rg * Wp].rearrange("p (q t) -> p q t", t=2 * Wp)
                    in0 = rows[:, :, 1 : 1 + W]
                    in1 = rows[:, :, Wp + 1 : Wp + 1 + W]
                    q0, q1 = r0 // 2, (r0 + rg) // 2
                    q = rg // 2
                    prA = pr_p.tile(
                        [KP, max_pairs, W], F32, tag="prA", bufs=1,
                        name=f"prA{img}{si}{j}",
                    )
                    nc.scalar.copy(prA[:, :q, :], in0)
                    nc.vector.tensor_max(
                        pr_tiles[j][:, q0:q1, :], prA[:, :q, :], in1
                    )
                    # pool step 2 + BN2 for this stretch's rows
                    prs = pr_tiles[j][:, q0:q1, :].rearrange("p q w -> p (q w)")
                    pv = pooled_tiles[j].rearrange("p (q w) -> p q w", w=WH)[
                        :, q0:q1, :
                    ]
                    nc.vector.tensor_max(pv, prs[:, 0::2], prs[:, 1::2])
                    nc.vector.tensor_scalar(
                        pv, pv, s2sb[:, j : j + 1], b2sb[:, j : j + 1],
                        Alu.mult, Alu.add,
                    )
                    if j == NCHUNK - 1:
                        # transpose every output block fully covered now
                        b0 = (q0 * WH + OB - 1) // OB
                        b1 = (q1 * WH) // OB
                        for b in range(b0, b1):
                            otp = tp_p.tile(
                                [OB, NCHUNK, KP], F32, tag="tp",
                                name=f"tpo{img}{b}",
                            )
                            for jj in range(NCHUNK):
                                nc.tensor.transpose(
                                    otp[:, jj, :],
                                    pooled_tiles[jj][:, OB * b : OB * (b + 1)],
                                    id32sb[:, :],
                                )
                            nc.scalar.copy(
                                onat[:, img - g0, b, :],
                                otp[:, :, :].rearrange("p a b -> p (a b)"),
                            )

                conv(hsT, w2sb, conv2_st, pool1)

                if img == g0 + gn - 1:
                    if SPLIT_LAST and img == B - 1:
                        # ship the early blocks mid-image, the rest at the end
                        cut = NOB // 2
                        for blo, bhi in ((0, cut), (cut, NOB)):
                            dst = dram_ap(
                                y_h,
                                (g0 * NOB + blo) * OB * C,
                                [[C, OB], [OB * C, (bhi - blo)], [1, C]],
                            )
                            nc.sync.dma_start(
                                out=dst,
                                in_=onat[:, 0, blo:bhi, :].rearrange(
                                    "p b c -> p b c"
                                ),
                            )
                    else:
                        dst = dram_ap(
                            y_h, g0 * PO * C, [[C, OB], [OB * C, gn * NOB], [1, C]]
                        )
                        nc.sync.dma_start(
                            out=dst,
                            in_=onat[:, :gn, :, :].rearrange("p a b c -> p (a b) c"),
                        )

            # software-pipelined emission: input prep leads convs by one image
            prep_input(0)
            load_consts()
            for img in range(B):
                if img + 1 < B:
                    prep_input(img + 1)
                run_convs(img)

    nc.compile()
    return nc


# ---------------------------------------------------------------------------
# host-side constant prep
# ---------------------------------------------------------------------------


def _prep_consts(w1, beta1, mean1, var1, w2, beta2, mean2, var2):
    import jax
    import jax.numpy as jnp
    from jax import lax
    from concourse import mybir

    fp8np = mybir.dt.np(mybir.dt.float8e4)

    def prep_w(w):
        ws = np.where(np.asarray(w) >= 0, np.float32(1.0), np.float32(-1.0))
        # [3,3,ci,co] -> [p, tap, j, ktile, m]; ci = ktile*128+p, co = j*128+m
        wr = ws.reshape(9, 2, KP, NCHUNK, KP).transpose(2, 0, 3, 1, 4)
        return np.ascontiguousarray(wr).astype(fp8np)

    w1p, w2p = prep_w(w1), prep_w(w2)

    cpu = jax.devices("cpu")[0]
    MAXH = 9 * C
    with jax.default_device(cpu):
        hs = jnp.arange(-MAXH, MAXH + 1, dtype=jnp.float32)
        bn1 = (hs[:, None] - jnp.asarray(mean1)[None, :]) * lax.rsqrt(
            jnp.asarray(var1) + 1e-3
        )[None, :] + jnp.asarray(beta1)[None, :]
        nonneg = np.asarray(bn1 >= 0)
        r2 = np.asarray(lax.rsqrt(jnp.asarray(var2) + 1e-3))

    assert (np.diff(nonneg.astype(np.int8), axis=0) >= 0).all(), "bn1 not monotone"
    kc = np.where(nonneg.any(0), nonneg.argmax(0), 2 * MAXH + 1) - MAXH
    # device psum holds h/2 (x=+-0.5, w=+-1): sign flips at (kc-0.5)/2
    nt1 = (-(kc.astype(np.float64) - 0.5) / 2.0).astype(np.float32)

    s2 = r2.astype(np.float32)
    b2 = (
        np.asarray(beta2, np.float64)
        - np.asarray(mean2, np.float64) * s2.astype(np.float64)
    ).astype(np.float32)

    def to_pj(a):  # [256] -> [128, 2] with c = j*128+p
        return np.ascontiguousarray(a.reshape(NCHUNK, KP).T).astype(np.float32)

    # pack everything into one [128, CONST_B] uint8 image
    cbuf = np.zeros((KP, CONST_B), dtype=np.uint8)

    def put(off, arr):
        by = np.ascontiguousarray(arr).reshape(KP, -1).view(np.uint8)
        cbuf[:, off : off + by.shape[1]] = by

    put(W1_OFF, w1p)
    put(W2_OFF, w2p)
    put(NT1_OFF, to_pj(nt1))
    put(S2_OFF, to_pj(s2))
    put(B2_OFF, to_pj(b2))
    return {"cb": cbuf}


# ---------------------------------------------------------------------------
# entry point
# ---------------------------------------------------------------------------

_cached = {}


def _run(inputs, trace=False):
    from concourse import bass_utils

    x = np.asarray(inputs["x"], dtype=np.float32)
    Bt, H, W, _ = x.shape  # 32, 56, 56, 256
    Bc = Bt // N_CORES

    consts = _prep_consts(
        inputs["w1"], inputs["beta1"], inputs["mean1"], inputs["var1"],
        inputs["w2"], inputs["beta2"], inputs["mean2"], inputs["var2"],
    )

    key = (Bc, H, W)
    if key not in _cached:
        _cached[key] = build_program(Bc, H, W)
    nc = _cached[key]

    in_maps = []
    for c in range(N_CORES):
        m = dict(consts)
        m["x"] = np.ascontiguousarray(x[c * Bc : (c + 1) * Bc].reshape(Bc, H * W, C))
        in_maps.append(m)

    res = bass_utils.run_bass_kernel_spmd(
        nc, in_maps, core_ids=list(range(N_CORES)), trace=trace
    )
    y = np.concatenate([r["y"] for r in res.results], axis=0)
    y = y.reshape(Bt, H // 2, W // 2, C).astype(np.float32)
    return y, res


def kernel(**inputs):
    y, _ = _run(inputs, trace=False)
    return y



# revision 6
# speedup vs baseline: 1.1970x; 1.1970x over previous
"""Trainium2 Bass kernel for a BinaryNet conv block.

Pipeline (per core, data-parallel over batch):
  sign(x) -> conv3x3(sign(w1)) -> BN1 -> sign -> conv3x3(sign(w2))
          -> maxpool2x2 -> BN2

Implementation notes:
  - Activations are +-0.5, weights +-1.0 in fp8e4 (exactly representable);
    convs run as 9 shifted-window matmuls with DoubleRow perf mode (K=256
    contraction per instruction), accumulating exactly into fp32 PSUM.
  - BN1+sign is fused into one ScalarE Sign activation against a
    host-precomputed per-channel threshold. Conv outputs are exact
    integers, so an integer cutoff k_c reproduces the reference's fp32
    sign decisions bit-exactly.
  - The host marshals x to channel-major [C, H*W] per image and reads y
    back channel-major [2, 128, PO]; the device never transposes. The PE
    therefore runs conv matmuls only, fed by DVE sign + ACT/DVE scatter
    copies into the zero-bordered padded layout.
  - Spatial layout is channel-major [ci, y*(W+2)+x] with a zero border so
    the 9 taps are just constant AP offsets.
  - The bass2jax/pseudo-DMA path allows only ONE sync wait per DMA; every
    DMA destination is a fresh tile (or a disjoint slice of one), so no
    DMA ever needs more than one semaphore wait. All loads are issued
    up-front in priority order (DMA transfers serialize), stores as
    produced.
  - A short burst of junk transposes warms the PE p-state ramp so the
    first real conv matmuls run at full clock.
"""

import os
import numpy as np

os.environ.setdefault("MYCRO_LOCAL_CACHE", "1")

N_CORES = 8
C = 256
NCHUNK = 2  # channel chunks of 128
KP = 128

# packed consts layout (bytes per partition)
W1_OFF = 0          # fp8 [9,2,2,128] -> 4608 B
NT1_OFF = 4608      # f32 [2] -> 8 B
CB1_B = 4616        # first consts DMA covers [0, CB1_B)
W2_OFF = 4616       # fp8 -> 4608 B
S2_OFF = 9224       # f32 [2]
B2_OFF = 9232       # f32 [2]
CONST_B = 9240


def build_program(B, H, W, psum_stretch=1024, conv_bufs=3, warm_mm=100):
    """Build the per-core Bass program. B images of HxWxC per core."""
    import concourse.bass as bass
    import concourse.bacc as bacc
    import concourse.tile as tile
    from concourse import mybir

    F32 = mybir.dt.float32
    FP8 = mybir.dt.float8e4
    BF16 = mybir.dt.bfloat16
    U8 = mybir.dt.uint8
    DR = mybir.MatmulPerfMode.DoubleRow
    Alu = mybir.AluOpType
    Act = mybir.ActivationFunctionType

    Hp, Wp = H + 2, W + 2
    S_pad = Hp * Wp
    S = H * W
    DOFF = 32  # left zero pad inside each channel-chunk row buffer
    S_chunk = ((S_pad + DOFF + 32 + 15) // 16) * 16  # right pad >= 32
    NQ = 4  # prep groups (and img-0 load quarters) per image
    GR = H // NQ  # rows per prep group
    assert H % NQ == 0
    PO = (H // 2) * (W // 2)
    WH = W // 2

    # conv2 row groups (pool-pair aligned)
    max_rows = (psum_stretch // Wp) // 2 * 2
    row_groups = []
    r = 0
    while r < H:
        g = min(max_rows, H - r)
        row_groups.append((r, g))
        r += g
    st2 = [((1 + r0) * Wp, rg * Wp, r0, rg) for r0, rg in row_groups]
    # conv1 row groups for image 0: aligned so stretch s only needs input
    # rows < GR*(s+1) (quarter-granular load pipelining at startup)
    rg1 = []
    r = 0
    for s in range(NQ):
        hi = min(GR * (s + 1) - 1, H)
        if s == NQ - 1:
            hi = H
        rg1.append((r, hi - r))
        r = hi
    st1_first = [((1 + r0) * Wp, rg * Wp, r0, rg) for r0, rg in rg1]
    st1_rest = st2
    PS_COLS = psum_stretch

    nc = bacc.Bacc("TRN2", target_bir_lowering=False, debug=False)

    x_h = nc.dram_tensor("x", [B, C, S], F32, kind="ExternalInput")
    cb_h = nc.dram_tensor("cb", [KP, CONST_B], U8, kind="ExternalInput")
    y_h = nc.dram_tensor("y", [B, NCHUNK, KP, PO], F32, kind="ExternalOutput")

    def dram_ap(handle, offset, dims):
        return bass.AP(
            tensor=handle.ap().tensor, offset=offset, ap=[list(d) for d in dims]
        )

    with tile.TileContext(nc) as tc:
        from contextlib import ExitStack

        with ExitStack() as ctx:
            consts = ctx.enter_context(tc.tile_pool(name="consts", bufs=1))
            xnat_p = ctx.enter_context(tc.tile_pool(name="xnat", bufs=1))
            xsg_p = ctx.enter_context(tc.tile_pool(name="xsg", bufs=2))
            xsT_p = ctx.enter_context(tc.tile_pool(name="xsT", bufs=2))
            hsT_p = ctx.enter_context(tc.tile_pool(name="hsT", bufs=2))
            pr_p = ctx.enter_context(tc.tile_pool(name="prp", bufs=2))
            po_p = ctx.enter_context(tc.tile_pool(name="pop", bufs=2))
            convp = ctx.enter_context(
                tc.tile_pool(name="convp", bufs=conv_bufs, space="PSUM")
            )
            warm_p = ctx.enter_context(tc.tile_pool(name="warm", bufs=1, space="PSUM"))

            # --- packed constants (two DMAs: w1+nt1 early, rest later)
            cb = consts.tile([KP, CONST_B], U8)
            w1sb = cb[:, W1_OFF : W1_OFF + 4608].bitcast(FP8).rearrange(
                "p (t j k m) -> p t j k m", t=9, j=NCHUNK, k=2
            )
            w2sb = cb[:, W2_OFF : W2_OFF + 4608].bitcast(FP8).rearrange(
                "p (t j k m) -> p t j k m", t=9, j=NCHUNK, k=2
            )
            nt1sb = cb[:, NT1_OFF : NT1_OFF + 8].bitcast(F32)
            s2sb = cb[:, S2_OFF : S2_OFF + 8].bitcast(F32)
            b2sb = cb[:, B2_OFF : B2_OFF + 8].bitcast(F32)

            from concourse import masks

            id8sb = consts.tile([KP, KP], BF16)
            masks.make_identity(nc, id8sb)

            # --- PE p-state warmup: dependency-free junk transposes keep the
            # tensor engine busy from t~0 so the ramp is spent before real
            # conv matmuls arrive.
            warm = warm_p.tile([KP, KP], BF16, tag="warm", name="warm")
            for _ in range(warm_mm):
                nc.tensor.transpose(warm, id8sb, id8sb)

            # --- loads, issued in priority order (DMA transfers serialize)
            xn = {}
            for img in range(B):
                xn[img] = xnat_p.tile(
                    [KP, NCHUNK, S], F32, tag=f"xn{img}", name=f"xn{img}"
                )

            def load_x_span(img, s0, s1):
                nc.sync.dma_start(
                    out=xn[img][:, :, s0:s1],
                    in_=dram_ap(
                        x_h,
                        img * C * S + s0,
                        [[S, KP], [KP * S, NCHUNK], [1, s1 - s0]],
                    ),
                )

            Q = GR * W  # spatial elems per quarter
            load_x_span(0, 0, Q)
            load_x_span(0, Q, 2 * Q)
            nc.sync.dma_start(out=cb[:, :CB1_B], in_=cb_h.ap()[:, :CB1_B])
            load_x_span(0, 2 * Q, 3 * Q)
            load_x_span(0, 3 * Q, 4 * Q)
            nc.sync.dma_start(out=cb[:, CB1_B:], in_=cb_h.ap()[:, CB1_B:])
            for img in range(1, B):
                load_x_span(img, 0, S // 2)
                load_x_span(img, S // 2, S)

            # --- helpers
            def border_memsets(buf):
                # rows 0 and H+1, left/right pads, and border cols {0, W+1} of
                # rows 1..H; on GPSIMD so the vector engines stay free.
                nc.gpsimd.memset(buf[:, :, 0 : DOFF + Wp], 0.0)
                nc.gpsimd.memset(buf[:, :, DOFF + (H + 1) * Wp : S_chunk], 0.0)
                rows = buf[:, :, DOFF + Wp : DOFF + (H + 1) * Wp].rearrange(
                    "p j (r w) -> p j r w", w=Wp
                )
                nc.gpsimd.memset(rows[:, :, :, 0 :: (W + 1)], 0.0)

            xs_tiles = {}
            xsT_tiles = {}

            def sign_group(img, g):
                # sign of quarter g: contiguous fp32 -> bf16 +-0.5 on DVE
                if g == 0:
                    xs_tiles[img] = xsg_p.tile(
                        [KP, NCHUNK, S], BF16, tag="xs", name=f"xs{img}"
                    )
                nc.vector.tensor_scalar(
                    xs_tiles[img][:, :, g * Q : (g + 1) * Q],
                    xn[img][:, :, g * Q : (g + 1) * Q],
                    0.0, 0.5, Alu.is_ge, Alu.subtract,
                )

            def copy_group(img, g):
                # scatter the signed quarter into the padded fp8 conv layout;
                # j0 on ACT, j1 on DVE so the two run in parallel
                if g == 0:
                    xsT_tiles[img] = xsT_p.tile(
                        [KP, NCHUNK, S_chunk], FP8, tag="xsT", name=f"xsT{img}"
                    )
                    border_memsets(xsT_tiles[img])
                xsT = xsT_tiles[img]
                a0 = DOFF + (1 + GR * g) * Wp
                for j in range(NCHUNK):
                    src = xs_tiles[img][:, j, g * Q : (g + 1) * Q].rearrange(
                        "p (r w) -> p r w", w=W
                    )
                    dst = xsT[:, j, a0 : a0 + GR * Wp].rearrange(
                        "p (r w) -> p r w", w=Wp
                    )[:, :, 1 : 1 + W]
                    if j == 0:
                        nc.scalar.copy(dst, src)
                    else:
                        nc.vector.tensor_copy(dst, src)

            def conv_stretch(inbuf, wsb, st, si, j, psum_cb, nm):
                cs, cn = st[0], st[1]
                ps = convp.tile([KP, PS_COLS], F32, tag="cv", name=f"cv{nm}{si}{j}")
                for t in range(9):
                    dy, dx = t // 3, t % 3
                    off = (dy - 1) * Wp + (dx - 1)
                    lhsT = wsb[:, t, j]
                    for c0 in range(0, cn, 512):
                        n = min(512, cn - c0)
                        a = DOFF + cs + off + c0
                        nc.tensor.matmul(
                            ps[:, c0 : c0 + n],
                            lhsT,
                            inbuf[:, :, a : a + n],
                            start=(t == 0),
                            stop=(t == 8),
                            perf_mode=DR,
                        )
                psum_cb(si, j, ps, st)

            hsT_tiles = {}

            def conv1_stretch(img, si):
                st = (st1_first if img == 0 else st1_rest)[si]
                if si == 0:
                    hsT_tiles[img] = hsT_p.tile(
                        [KP, NCHUNK, S_chunk], FP8, tag="hsT", name=f"hsT{img}"
                    )
                    border_memsets(hsT_tiles[img])
                hsT = hsT_tiles[img]

                def bnsign(si_, j, ps, st_):
                    cs, cn = st_[0], st_[1]
                    dstv = hsT[:, j, DOFF + cs : DOFF + cs + cn].rearrange(
                        "p (r w) -> p r w", w=Wp
                    )[:, :, 1 : 1 + W]
                    srcv = ps[:, :cn].rearrange("p (r w) -> p r w", w=Wp)[
                        :, :, 1 : 1 + W
                    ]
                    nc.scalar.activation(
                        dstv, srcv, Act.Sign, bias=nt1sb[:, j : j + 1], scale=1.0
                    )

                for j in range(NCHUNK):
                    conv_stretch(xsT_tiles[img], w1sb, st, si, j, bnsign, f"a{img}")
                if si == len(st1_rest) - 1:
                    xsT_tiles.pop(img)

            pr_tiles = {}
            pooled_tiles = {}

            def conv2_stretch(img, si):
                st = st2[si]
                if si == 0:
                    pr_tiles[img] = [
                        pr_p.tile([KP, H // 2, W], F32, tag="pr", name=f"pr{img}{j}")
                        for j in range(NCHUNK)
                    ]
                    pooled_tiles[img] = [
                        po_p.tile([KP, PO], F32, tag="pooled", name=f"pl{img}{j}")
                        for j in range(NCHUNK)
                    ]

                def pool_cb(si_, j, ps, st_):
                    cs, cn, r0, rg = st_
                    rows = ps[:, : rg * Wp].rearrange("p (q t) -> p q t", t=2 * Wp)
                    in0 = rows[:, :, 1 : 1 + W]
                    in1 = rows[:, :, Wp + 1 : Wp + 1 + W]
                    q0, q1 = r0 // 2, (r0 + rg) // 2
                    q = rg // 2
                    # TensorTensor may read only one input from PSUM: stage
                    # the even rows into SBUF on ACT, max against PSUM on DVE
                    prA = pr_p.tile(
                        [KP, max_rows // 2, W], F32, tag="prA", bufs=1,
                        name=f"prA{img}{si_}{j}",
                    )
                    nc.scalar.copy(prA[:, :q, :], in0)
                    nc.vector.tensor_max(
                        pr_tiles[img][j][:, q0:q1, :], prA[:, :q, :], in1
                    )
                    prs = pr_tiles[img][j][:, q0:q1, :].rearrange("p q w -> p (q w)")
                    pv = pooled_tiles[img][j].rearrange("p (q w) -> p q w", w=WH)[
                        :, q0:q1, :
                    ]
                    nc.vector.tensor_max(pv, prs[:, 0::2], prs[:, 1::2])
                    nc.vector.tensor_scalar(
                        pv, pv, s2sb[:, j : j + 1], b2sb[:, j : j + 1],
                        Alu.mult, Alu.add,
                    )
                    # stores: whole channel-chunk per image, but per-stretch
                    # for the last image so the tail ships immediately
                    if img == B - 1:
                        nc.sync.dma_start(
                            out=dram_ap(
                                y_h,
                                (img * NCHUNK + j) * KP * PO + q0 * WH,
                                [[PO, KP], [1, (q1 - q0) * WH]],
                            ),
                            in_=pooled_tiles[img][j][:, q0 * WH : q1 * WH],
                        )
                    elif si_ == len(st2) - 1:
                        nc.sync.dma_start(
                            out=dram_ap(
                                y_h,
                                (img * NCHUNK + j) * KP * PO,
                                [[PO, KP], [1, PO]],
                            ),
                            in_=pooled_tiles[img][j],
                        )

                for j in range(NCHUNK):
                    conv_stretch(hsT_tiles[img], w2sb, st, si, j, pool_cb, f"b{img}")
                if si == len(st2) - 1:
                    hsT_tiles.pop(img)

            # --- emission ---
            # image 0: prep quarters run one group ahead of its conv1
            # stretches so ACT copies are never queued behind bnsigns
            sign_group(0, 0)
            copy_group(0, 0)
            for g in range(NQ):
                if g + 1 < NQ:
                    sign_group(0, g + 1)
                    copy_group(0, g + 1)
                conv1_stretch(0, g)
            for img in range(B):
                if img > 0:
                    for si in range(len(st2)):
                        conv1_stretch(img, si)
                        if img + 1 < B:
                            sign_group(img + 1, si)
                for si in range(len(st2)):
                    if img == 0 and B > 1:
                        sign_group(1, si)
                    conv2_stretch(img, si)
                    if img + 1 < B:
                        copy_group(img + 1, si)

    nc.compile()
    return nc


# ---------------------------------------------------------------------------
# host-side constant prep
# ---------------------------------------------------------------------------


def _prep_consts(w1, beta1, mean1, var1, w2, beta2, mean2, var2):
    import jax
    import jax.numpy as jnp
    from jax import lax
    from concourse import mybir

    fp8np = mybir.dt.np(mybir.dt.float8e4)

    def prep_w(w):
        ws = np.where(np.asarray(w) >= 0, np.float32(1.0), np.float32(-1.0))
        # [3,3,ci,co] -> [p, tap, j, ktile, m]; ci = ktile*128+p, co = j*128+m
        wr = ws.reshape(9, 2, KP, NCHUNK, KP).transpose(2, 0, 3, 1, 4)
        return np.ascontiguousarray(wr).astype(fp8np)

    w1p, w2p = prep_w(w1), prep_w(w2)

    cpu = jax.devices("cpu")[0]
    MAXH = 9 * C
    with jax.default_device(cpu):
        hs = jnp.arange(-MAXH, MAXH + 1, dtype=jnp.float32)
        bn1 = (hs[:, None] - jnp.asarray(mean1)[None, :]) * lax.rsqrt(
            jnp.asarray(var1) + 1e-3
        )[None, :] + jnp.asarray(beta1)[None, :]
        nonneg = np.asarray(bn1 >= 0)
        r2 = np.asarray(lax.rsqrt(jnp.asarray(var2) + 1e-3))

    assert (np.diff(nonneg.astype(np.int8), axis=0) >= 0).all(), "bn1 not monotone"
    kc = np.where(nonneg.any(0), nonneg.argmax(0), 2 * MAXH + 1) - MAXH
    # device psum holds h/2 (x=+-0.5, w=+-1): sign flips at (kc-0.5)/2
    nt1 = (-(kc.astype(np.float64) - 0.5) / 2.0).astype(np.float32)

    s2 = r2.astype(np.float32)
    b2 = (
        np.asarray(beta2, np.float64)
        - np.asarray(mean2, np.float64) * s2.astype(np.float64)
    ).astype(np.float32)

    def to_pj(a):  # [256] -> [128, 2] with c = j*128+p
        return np.ascontiguousarray(a.reshape(NCHUNK, KP).T).astype(np.float32)

    # pack everything into one [128, CONST_B] uint8 image
    cbuf = np.zeros((KP, CONST_B), dtype=np.uint8)

    def put(off, arr):
        by = np.ascontiguousarray(arr).reshape(KP, -1).view(np.uint8)
        cbuf[:, off : off + by.shape[1]] = by

    put(W1_OFF, w1p)
    put(W2_OFF, w2p)
    put(NT1_OFF, to_pj(nt1))
    put(S2_OFF, to_pj(s2))
    put(B2_OFF, to_pj(b2))
    return {"cb": cbuf}


# ---------------------------------------------------------------------------
# entry point
# ---------------------------------------------------------------------------

_cached = {}


def _run(inputs, trace=False):
    from concourse import bass_utils

    x = np.asarray(inputs["x"], dtype=np.float32)
    Bt, H, W, _ = x.shape  # 32, 56, 56, 256
    Bc = Bt // N_CORES

    consts = _prep_consts(
        inputs["w1"], inputs["beta1"], inputs["mean1"], inputs["var1"],
        inputs["w2"], inputs["beta2"], inputs["mean2"], inputs["var2"],
    )

    key = (Bc, H, W)
    if key not in _cached:
        _cached[key] = build_program(Bc, H, W)
    nc = _cached[key]

    # channel-major marshaling: [Bt, H*W, C] -> per-core [Bc, C, H*W]
    xcm = np.ascontiguousarray(
        x.reshape(Bt, H * W, C).transpose(0, 2, 1)
    )

    in_maps = []
    for c in range(N_CORES):
        m = dict(consts)
        m["x"] = xcm[c * Bc : (c + 1) * Bc]
        in_maps.append(m)

    res = bass_utils.run_bass_kernel_spmd(
        nc, in_maps, core_ids=list(range(N_CORES)), trace=trace
    )
    PO = (H // 2) * (W // 2)
    # y comes back channel-major [Bc, 2, 128, PO]; restore NHWC
    y = np.concatenate(
        [r["y"].reshape(Bc, C, PO).transpose(0, 2, 1) for r in res.results], axis=0
    )
    y = np.ascontiguousarray(y.reshape(Bt, H // 2, W // 2, C)).astype(np.float32)
    return y, res


def kernel(**inputs):
    y, _ = _run(inputs, trace=False)
    return y


# revision 40
# speedup vs baseline: 1.2446x; 1.0397x over previous
"""Trainium2 Bass kernel for a BinaryNet conv block.

Pipeline (per core, data-parallel over batch):
  sign(x) -> conv3x3(sign(w1)) -> BN1 -> sign -> conv3x3(sign(w2))
          -> maxpool2x2 -> BN2

Implementation notes:
  - Activations are +-0.5, weights +-1.0 in fp8e4 (exactly representable);
    convs run as 9 shifted-window matmuls with DoubleRow perf mode (K=256
    contraction per instruction), accumulating exactly into fp32 PSUM.
  - BN1+sign is fused into one ScalarE Sign activation against a
    host-precomputed per-channel threshold. Conv outputs are exact
    integers, so an integer cutoff k_c reproduces the reference's fp32
    sign decisions bit-exactly.
  - The host marshals x to channel-major [C, H*W] per image and reads y
    back channel-major [2, 128, PO]; the device never transposes. The PE
    therefore runs conv matmuls only, fed by DVE sign + ACT/DVE scatter
    copies into the zero-bordered padded layout.
  - Spatial layout is channel-major [ci, y*(W+2)+x] with a zero border so
    the 9 taps are just constant AP offsets.
  - The bass2jax/pseudo-DMA path allows only ONE sync wait per DMA; every
    DMA destination is a fresh tile (or a disjoint slice of one), so no
    DMA ever needs more than one semaphore wait. All loads are issued
    up-front in priority order (DMA transfers serialize), stores as
    produced.
  - A short burst of junk transposes warms the PE p-state ramp so the
    first real conv matmuls run at full clock.
"""

import os
import numpy as np

os.environ.setdefault("MYCRO_LOCAL_CACHE", "1")

N_CORES = 8
C = 256
NCHUNK = 2  # channel chunks of 128
KP = 128

# packed consts layout (bytes per partition); w1 split by output-channel
# chunk so the first conv can start as soon as the j0 half lands
W1J0_OFF = 0        # fp8 [9,2,128] -> 2304 B
NT1_OFF = 2304      # f32 [2] -> 8 B
CBA_B = 2312        # first consts DMA covers [0, CBA_B)
W1J1_OFF = 2312     # fp8 -> 2304 B
CBB_B = 4616        # second consts DMA covers [CBA_B, CBB_B)
W2_OFF = 4616       # fp8 [9,2,2,128] -> 4608 B
S2_OFF = 9224       # f32 [2]
B2_OFF = 9232       # f32 [2]
CONST_B = 9240


def build_program(B, H, W, psum_stretch=1024, conv_bufs=4, warm_mm=60, tail_split=0):
    """Build the per-core Bass program. B images of HxWxC per core."""
    import concourse.bass as bass
    import concourse.bacc as bacc
    import concourse.tile as tile
    from concourse import mybir

    F32 = mybir.dt.float32
    FP8 = mybir.dt.float8e4
    BF16 = mybir.dt.bfloat16
    U8 = mybir.dt.uint8
    DR = mybir.MatmulPerfMode.DoubleRow
    Alu = mybir.AluOpType
    Act = mybir.ActivationFunctionType

    Hp, Wp = H + 2, W + 2
    S_pad = Hp * Wp
    S = H * W
    DOFF = 32  # left zero pad inside each channel-chunk row buffer
    S_chunk = ((S_pad + DOFF + 32 + 15) // 16) * 16  # right pad >= 32
    NQ = 4  # prep groups (and img-0 load quarters) per image
    GR = H // NQ  # rows per prep group
    assert H % NQ == 0
    PO = (H // 2) * (W // 2)
    WH = W // 2

    # conv2 row groups (pool-pair aligned)
    max_rows = (psum_stretch // Wp) // 2 * 2
    row_groups = []
    r = 0
    while r < H:
        g = min(max_rows, H - r)
        row_groups.append((r, g))
        r += g
    st2 = [((1 + r0) * Wp, rg * Wp, r0, rg) for r0, rg in row_groups]
    # last image: split the final row group so the pool/store chain after
    # the very last matmul is as short as possible
    tail_rows = list(row_groups[:-1])
    lr0, lrg = row_groups[-1]
    if tail_split and lrg > tail_split:
        tail_rows += [(lr0, lrg - tail_split), (lr0 + lrg - tail_split, tail_split)]
    else:
        tail_rows.append((lr0, lrg))
    st2_tail = [((1 + r0) * Wp, rg * Wp, r0, rg) for r0, rg in tail_rows]
    # image 0 is prepped in 7-row pieces (NP1 of them) with conv1 stretches
    # aligned so stretch s only reads input rows loaded by pieces <= s. The
    # +Wp+1 shifted window spills one byte into the row after r0+rg, so
    # reserve one extra row per boundary.
    GR1 = GR  # rows per image-0 prep piece
    NP1 = H // GR1
    rg1 = []
    r = 0
    for s in range(NP1):
        hi = GR1 * (s + 1) - 2
        if s == NP1 - 1:
            hi = H
        rg1.append((r, hi - r))
        r = hi
    st1_first = [((1 + r0) * Wp, rg * Wp, r0, rg) for r0, rg in rg1]
    st1_rest = st2
    PS_COLS = psum_stretch

    nc = bacc.Bacc("TRN2", target_bir_lowering=False, debug=False)

    x_h = nc.dram_tensor("x", [B, C, S], F32, kind="ExternalInput")
    cb_h = nc.dram_tensor("cb", [KP, CONST_B], U8, kind="ExternalInput")
    y_h = nc.dram_tensor("y", [B, NCHUNK, KP, PO], F32, kind="ExternalOutput")

    def dram_ap(handle, offset, dims):
        return bass.AP(
            tensor=handle.ap().tensor, offset=offset, ap=[list(d) for d in dims]
        )

    with tile.TileContext(nc) as tc:
        from contextlib import ExitStack

        with ExitStack() as ctx:
            consts = ctx.enter_context(tc.tile_pool(name="consts", bufs=1))
            xnat_p = ctx.enter_context(tc.tile_pool(name="xnat", bufs=1))
            xsT_p = ctx.enter_context(tc.tile_pool(name="xsT", bufs=2))
            hsT_p = ctx.enter_context(tc.tile_pool(name="hsT", bufs=2))
            pr_p = ctx.enter_context(tc.tile_pool(name="prp", bufs=2))
            po_p = ctx.enter_context(tc.tile_pool(name="pop", bufs=2))
            convp = ctx.enter_context(
                tc.tile_pool(name="convp", bufs=conv_bufs, space="PSUM")
            )

            # --- packed constants (three DMAs: w1-j0+nt1, w1-j1, rest)
            cb = consts.tile([KP, CONST_B], U8)
            w1j = [
                cb[:, W1J0_OFF : W1J0_OFF + 2304].bitcast(FP8).rearrange(
                    "p (t k m) -> p t k m", t=9, k=2
                ),
                cb[:, W1J1_OFF : W1J1_OFF + 2304].bitcast(FP8).rearrange(
                    "p (t k m) -> p t k m", t=9, k=2
                ),
            ]
            w2sb = cb[:, W2_OFF : W2_OFF + 4608].bitcast(FP8).rearrange(
                "p (t j k m) -> p t j k m", t=9, j=NCHUNK, k=2
            )
            w1v = lambda j, t: w1j[j][:, t]
            w2v = lambda j, t: w2sb[:, t, j]
            nt1sb = cb[:, NT1_OFF : NT1_OFF + 8].bitcast(F32)
            s2sb = cb[:, S2_OFF : S2_OFF + 8].bitcast(F32)
            b2sb = cb[:, B2_OFF : B2_OFF + 8].bitcast(F32)

            # --- preload the ACT piecewise-poly table (Sign) with a tiny
            # dependency-free activation so the 1.3us table load is off the
            # critical prep chain
            dummy = consts.tile([1, 4], F32)
            nc.vector.memset(dummy, 0.0)
            nc.scalar.activation(dummy, dummy, Act.Sign, bias=0.0, scale=1.0)

            from concourse import masks

            id8sb = consts.tile([KP, KP], BF16)
            masks.make_identity(nc, id8sb)

            # --- PE p-state warmup: dependency-free junk transposes keep the
            # tensor engine busy from t~0 so the ramp is spent before real
            # conv matmuls arrive. The junk lives in a convp rotation buffer
            # (conv matmuls start=True overwrite it later).
            if warm_mm:
                warm = convp.tile([KP, KP], BF16, tag="cv", name="warm")
                for _ in range(warm_mm):
                    nc.tensor.transpose(warm, id8sb, id8sb)

            # --- loads, issued in priority order (DMA transfers serialize)
            xn = {}
            for img in range(B):
                xn[img] = xnat_p.tile(
                    [KP, NCHUNK, S], F32, tag=f"xn{img}", name=f"xn{img}"
                )

            def load_x_span(img, s0, s1):
                nc.sync.dma_start(
                    out=xn[img][:, :, s0:s1],
                    in_=dram_ap(
                        x_h,
                        img * C * S + s0,
                        [[S, KP], [KP * S, NCHUNK], [1, s1 - s0]],
                    ),
                )

            Q = GR * W   # spatial elems per steady-state prep quarter
            Q1 = GR1 * W  # spatial elems per image-0 piece
            load_x_span(0, 0, Q1)
            nc.sync.dma_start(out=cb[:, :CBA_B], in_=cb_h.ap()[:, :CBA_B])
            load_x_span(0, Q1, 2 * Q1)
            nc.sync.dma_start(out=cb[:, CBA_B:CBB_B], in_=cb_h.ap()[:, CBA_B:CBB_B])
            for k in range(2, NP1):
                load_x_span(0, k * Q1, (k + 1) * Q1)
            nc.sync.dma_start(out=cb[:, CBB_B:], in_=cb_h.ap()[:, CBB_B:])
            for img in range(1, B):
                load_x_span(img, 0, S // 2)
                load_x_span(img, S // 2, S)

            # --- helpers
            def border_memsets(buf):
                # rows 0 and H+1, left/right pads, and border cols {0, W+1} of
                # rows 1..H; on GPSIMD so the vector engines stay free.
                nc.gpsimd.memset(buf[:, :, 0 : DOFF + Wp], 0.0)
                nc.gpsimd.memset(buf[:, :, DOFF + (H + 1) * Wp : S_chunk], 0.0)
                rows = buf[:, :, DOFF + Wp : DOFF + (H + 1) * Wp].rearrange(
                    "p j (r w) -> p j r w", w=Wp
                )
                nc.gpsimd.memset(rows[:, :, :, 0 :: (W + 1)], 0.0)

            xsT_tiles = {}

            def prep_span(img, lo, hi):
                # fused sign+scatter of rows [lo, hi): fp32 -> fp8 +-0.5
                # written straight into the padded conv layout; j0 on DVE,
                # j1 on GPSIMD so the two planes run in parallel
                if lo == 0:
                    xsT_tiles[img] = xsT_p.tile(
                        [KP, NCHUNK, S_chunk], FP8, tag="xsT", name=f"xsT{img}"
                    )
                    border_memsets(xsT_tiles[img])
                xsT = xsT_tiles[img]
                a0 = DOFF + (1 + lo) * Wp
                for j in range(NCHUNK):
                    src = xn[img][:, j, lo * W : hi * W].rearrange(
                        "p (r w) -> p r w", w=W
                    )
                    dst = xsT[:, j, a0 : a0 + (hi - lo) * Wp].rearrange(
                        "p (r w) -> p r w", w=Wp
                    )[:, :, 1 : 1 + W]
                    eng = nc.vector if j == 0 else nc.gpsimd
                    eng.tensor_scalar(
                        dst, src, 0.0, 0.5, Alu.is_ge, Alu.subtract
                    )

            def prep_group(img, g):
                prep_span(img, g * GR, (g + 1) * GR)

            def conv_stretch(inbuf, wv, st, si, j, psum_cb, nm):
                cs, cn = st[0], st[1]
                ps = convp.tile([KP, PS_COLS], F32, tag="cv", name=f"cv{nm}{si}{j}")
                for t in range(9):
                    dy, dx = t // 3, t % 3
                    off = (dy - 1) * Wp + (dx - 1)
                    lhsT = wv(j, t)
                    for c0 in range(0, cn, 512):
                        n = min(512, cn - c0)
                        a = DOFF + cs + off + c0
                        nc.tensor.matmul(
                            ps[:, c0 : c0 + n],
                            lhsT,
                            inbuf[:, :, a : a + n],
                            start=(t == 0),
                            stop=(t == 8),
                            perf_mode=DR,
                        )
                psum_cb(si, j, ps, st)

            hsT_tiles = {}

            def conv1_stretch(img, si):
                sts1 = st1_first if img == 0 else st1_rest
                st = sts1[si]
                if si == 0:
                    hsT_tiles[img] = hsT_p.tile(
                        [KP, NCHUNK, S_chunk], FP8, tag="hsT", name=f"hsT{img}"
                    )
                    border_memsets(hsT_tiles[img])
                hsT = hsT_tiles[img]

                def bnsign(si_, j, ps, st_):
                    cs, cn = st_[0], st_[1]
                    dstv = hsT[:, j, DOFF + cs : DOFF + cs + cn].rearrange(
                        "p (r w) -> p r w", w=Wp
                    )[:, :, 1 : 1 + W]
                    srcv = ps[:, :cn].rearrange("p (r w) -> p r w", w=Wp)[
                        :, :, 1 : 1 + W
                    ]
                    nc.scalar.activation(
                        dstv, srcv, Act.Sign, bias=nt1sb[:, j : j + 1], scale=1.0
                    )

                for j in range(NCHUNK):
                    conv_stretch(xsT_tiles[img], w1v, st, si, j, bnsign, f"a{img}")
                if si == len(sts1) - 1:
                    xsT_tiles.pop(img)

            pr_tiles = {}
            pooled_tiles = {}

            def conv2_stretch(img, si):
                sts = st2_tail if img == B - 1 else st2
                st = sts[si]
                if si == 0:
                    pr_tiles[img] = [
                        pr_p.tile([KP, H // 2, W], F32, tag="pr", name=f"pr{img}{j}")
                        for j in range(NCHUNK)
                    ]
                    pooled_tiles[img] = [
                        po_p.tile([KP, PO], F32, tag="pooled", name=f"pl{img}{j}")
                        for j in range(NCHUNK)
                    ]

                def pool_cb(si_, j, ps, st_):
                    cs, cn, r0, rg = st_
                    rows = ps[:, : rg * Wp].rearrange("p (q t) -> p q t", t=2 * Wp)
                    in0 = rows[:, :, 1 : 1 + W]
                    in1 = rows[:, :, Wp + 1 : Wp + 1 + W]
                    q0, q1 = r0 // 2, (r0 + rg) // 2
                    q = rg // 2
                    # TensorTensor may read only one input from PSUM: stage
                    # the even rows into SBUF, then max against the PSUM odd
                    # rows. Steady state splits the copy to ACT for engine
                    # parallelism; the last image's short tail chains run
                    # entirely on DVE to avoid cross-engine sem hops.
                    prA = pr_p.tile(
                        [KP, max_rows // 2, W], F32, tag="prA", bufs=4,
                        name=f"prA{img}{si_}{j}",
                    )
                    nc.scalar.copy(prA[:, :q, :], in0)
                    nc.vector.tensor_max(
                        pr_tiles[img][j][:, q0:q1, :], prA[:, :q, :], in1
                    )
                    prs = pr_tiles[img][j][:, q0:q1, :].rearrange("p q w -> p (q w)")
                    pv = pooled_tiles[img][j].rearrange("p (q w) -> p q w", w=WH)[
                        :, q0:q1, :
                    ]
                    nc.vector.tensor_max(pv, prs[:, 0::2], prs[:, 1::2])
                    nc.vector.tensor_scalar(
                        pv, pv, s2sb[:, j : j + 1], b2sb[:, j : j + 1],
                        Alu.mult, Alu.add,
                    )
                    # stores: whole channel-chunk per image, but per-stretch
                    # for the last image so the tail ships immediately
                    if img == B - 1:
                        nc.sync.dma_start(
                            out=dram_ap(
                                y_h,
                                (img * NCHUNK + j) * KP * PO + q0 * WH,
                                [[PO, KP], [1, (q1 - q0) * WH]],
                            ),
                            in_=pooled_tiles[img][j][:, q0 * WH : q1 * WH],
                        )
                    elif si_ == len(sts) - 1:
                        nc.sync.dma_start(
                            out=dram_ap(
                                y_h,
                                (img * NCHUNK + j) * KP * PO,
                                [[PO, KP], [1, PO]],
                            ),
                            in_=pooled_tiles[img][j],
                        )

                for j in range(NCHUNK):
                    conv_stretch(hsT_tiles[img], w2v, st, si, j, pool_cb, f"b{img}")
                if si == len(sts) - 1:
                    hsT_tiles.pop(img)

            # --- emission ---
            # image 0: prep each 7-row piece right before the conv1 stretch
            # that needs it. The conv rhs spans both channel planes as one
            # interval hull, so any copy emitted before a stretch becomes a
            # dependency of it — never emit a copy ahead of an earlier
            # stretch.
            for k in range(NP1):
                prep_span(0, k * GR1, (k + 1) * GR1)
                conv1_stretch(0, k)
            for img in range(B):
                if img > 0:
                    for si in range(len(st2)):
                        conv1_stretch(img, si)
                        if img + 1 < B:
                            prep_group(img + 1, si)
                for si in range(len(st2_tail if img == B - 1 else st2)):
                    if img == 0 and B > 1 and si < NQ:
                        prep_group(1, si)
                    conv2_stretch(img, si)

    nc.compile()
    return nc


# ---------------------------------------------------------------------------
# host-side constant prep
# ---------------------------------------------------------------------------


def _prep_consts(w1, beta1, mean1, var1, w2, beta2, mean2, var2):
    import jax
    import jax.numpy as jnp
    from jax import lax
    from concourse import mybir

    fp8np = mybir.dt.np(mybir.dt.float8e4)

    def prep_w(w, j_major=False):
        ws = np.where(np.asarray(w) >= 0, np.float32(1.0), np.float32(-1.0))
        # [3,3,ci,co] -> [p, (j,) tap, ktile, m]; ci = ktile*128+p, co = j*128+m
        wr = ws.reshape(9, 2, KP, NCHUNK, KP)
        wr = wr.transpose((2, 3, 0, 1, 4) if j_major else (2, 0, 3, 1, 4))
        return np.ascontiguousarray(wr).astype(fp8np)

    w1p, w2p = prep_w(w1, j_major=True), prep_w(w2)

    cpu = jax.devices("cpu")[0]
    MAXH = 9 * C
    with jax.default_device(cpu):
        hs = jnp.arange(-MAXH, MAXH + 1, dtype=jnp.float32)
        bn1 = (hs[:, None] - jnp.asarray(mean1)[None, :]) * lax.rsqrt(
            jnp.asarray(var1) + 1e-3
        )[None, :] + jnp.asarray(beta1)[None, :]
        nonneg = np.asarray(bn1 >= 0)
        r2 = np.asarray(lax.rsqrt(jnp.asarray(var2) + 1e-3))

    assert (np.diff(nonneg.astype(np.int8), axis=0) >= 0).all(), "bn1 not monotone"
    kc = np.where(nonneg.any(0), nonneg.argmax(0), 2 * MAXH + 1) - MAXH
    # device psum holds h/2 (x=+-0.5, w=+-1): sign flips at (kc-0.5)/2
    nt1 = (-(kc.astype(np.float64) - 0.5) / 2.0).astype(np.float32)

    s2 = r2.astype(np.float32)
    b2 = (
        np.asarray(beta2, np.float64)
        - np.asarray(mean2, np.float64) * s2.astype(np.float64)
    ).astype(np.float32)

    def to_pj(a):  # [256] -> [128, 2] with c = j*128+p
        return np.ascontiguousarray(a.reshape(NCHUNK, KP).T).astype(np.float32)

    # pack everything into one [128, CONST_B] uint8 image
    cbuf = np.zeros((KP, CONST_B), dtype=np.uint8)

    def put(off, arr):
        by = np.ascontiguousarray(arr).reshape(KP, -1).view(np.uint8)
        cbuf[:, off : off + by.shape[1]] = by

    put(W1J0_OFF, w1p[:, 0])
    put(W1J1_OFF, w1p[:, 1])
    put(W2_OFF, w2p)
    put(NT1_OFF, to_pj(nt1))
    put(S2_OFF, to_pj(s2))
    put(B2_OFF, to_pj(b2))
    return {"cb": cbuf}


# ---------------------------------------------------------------------------
# entry point
# ---------------------------------------------------------------------------

_cached = {}


def _run(inputs, trace=False):
    from concourse import bass_utils

    x = np.asarray(inputs["x"], dtype=np.float32)
    Bt, H, W, _ = x.shape  # 32, 56, 56, 256
    Bc = Bt // N_CORES

    consts = _prep_consts(
        inputs["w1"], inputs["beta1"], inputs["mean1"], inputs["var1"],
        inputs["w2"], inputs["beta2"], inputs["mean2"], inputs["var2"],
    )

    key = (Bc, H, W)
    if key not in _cached:
        _cached[key] = build_program(Bc, H, W)
    nc = _cached[key]

    # channel-major marshaling: [Bt, H*W, C] -> per-core [Bc, C, H*W]
    xcm = np.ascontiguousarray(
        x.reshape(Bt, H * W, C).transpose(0, 2, 1)
    )

    in_maps = []
    for c in range(N_CORES):
        m = dict(consts)
        m["x"] = xcm[c * Bc : (c + 1) * Bc]
        in_maps.append(m)

    res = bass_utils.run_bass_kernel_spmd(
        nc, in_maps, core_ids=list(range(N_CORES)), trace=trace
    )
    PO = (H // 2) * (W // 2)
    # y comes back channel-major [Bc, 2, 128, PO]; restore NHWC
    y = np.concatenate(
        [r["y"].reshape(Bc, C, PO).transpose(0, 2, 1) for r in res.results], axis=0
    )
    y = np.ascontiguousarray(y.reshape(Bt, H // 2, W // 2, C)).astype(np.float32)
    return y, res


def kernel(**inputs):
    y, _ = _run(inputs, trace=False)
    return y


# revision 44
# speedup vs baseline: 1.2839x; 1.0316x over previous
"""Trainium2 Bass kernel for a BinaryNet conv block.

Pipeline (per core, data-parallel over batch):
  sign(x) -> conv3x3(sign(w1)) -> BN1 -> sign -> conv3x3(sign(w2))
          -> maxpool2x2 -> BN2

Implementation notes:
  - Activations are +-0.5, weights +-1.0 in fp8e4 (exactly representable);
    convs run as 9 shifted-window matmuls with DoubleRow perf mode (K=256
    contraction per instruction), accumulating exactly into fp32 PSUM.
  - BN1+sign is fused into one ScalarE Sign activation against a
    host-precomputed per-channel threshold. Conv outputs are exact
    integers, so an integer cutoff k_c reproduces the reference's fp32
    sign decisions bit-exactly.
  - The host marshals x to channel-major [C, H*W] per image and reads y
    back channel-major [2, 128, PO]; the device never transposes. The PE
    therefore runs conv matmuls only, fed by DVE sign + ACT/DVE scatter
    copies into the zero-bordered padded layout.
  - Spatial layout is channel-major [ci, y*(W+2)+x] with a zero border so
    the 9 taps are just constant AP offsets.
  - The bass2jax/pseudo-DMA path allows only ONE sync wait per DMA; every
    DMA destination is a fresh tile (or a disjoint slice of one), so no
    DMA ever needs more than one semaphore wait. All loads are issued
    up-front in priority order (DMA transfers serialize), stores as
    produced.
  - A short burst of junk transposes warms the PE p-state ramp so the
    first real conv matmuls run at full clock.
"""

import os
import numpy as np

os.environ.setdefault("MYCRO_LOCAL_CACHE", "1")

N_CORES = 8
C = 256
NCHUNK = 2  # channel chunks of 128
KP = 128

# packed consts layout (bytes per partition); w1 split by output-channel
# chunk so the first conv can start as soon as the j0 half lands
W1J0_OFF = 0        # fp8 [9,2,128] -> 2304 B
NT1_OFF = 2304      # f32 [2] -> 8 B
CBA_B = 2312        # first consts DMA covers [0, CBA_B)
W1J1_OFF = 2312     # fp8 -> 2304 B
CBB_B = 4616        # second consts DMA covers [CBA_B, CBB_B)
W2_OFF = 4616       # fp8 [9,2,2,128] -> 4608 B
S2_OFF = 9224       # f32 [2]
B2_OFF = 9232       # f32 [2]
CONST_B = 9240


def build_program(B, H, W, psum_stretch=1024, conv_bufs=4, warm_mm=60, tail_split=0, exact_rows=True):
    """Build the per-core Bass program. B images of HxWxC per core."""
    import concourse.bass as bass
    import concourse.bacc as bacc
    import concourse.tile as tile
    from concourse import mybir

    F32 = mybir.dt.float32
    FP8 = mybir.dt.float8e4
    BF16 = mybir.dt.bfloat16
    U8 = mybir.dt.uint8
    DR = mybir.MatmulPerfMode.DoubleRow
    Alu = mybir.AluOpType
    Act = mybir.ActivationFunctionType

    Hp, Wp = H + 2, W + 2
    S_pad = Hp * Wp
    S = H * W
    DOFF = 32  # left zero pad inside each channel-chunk row buffer
    S_chunk = ((S_pad + DOFF + 32 + 15) // 16) * 16  # right pad >= 32
    NQ = 4  # prep groups (and img-0 load quarters) per image
    GR = H // NQ  # rows per prep group
    assert H % NQ == 0
    PO = (H // 2) * (W // 2)
    WH = W // 2

    # conv2 row groups (pool-pair aligned)
    max_rows = (psum_stretch // Wp) // 2 * 2
    row_groups = []
    r = 0
    while r < H:
        g = min(max_rows, H - r)
        row_groups.append((r, g))
        r += g
    st2 = [((1 + r0) * Wp, rg * Wp, r0, rg) for r0, rg in row_groups]
    # last image: split the final row group so the pool/store chain after
    # the very last matmul is as short as possible
    tail_rows = list(row_groups[:-1])
    lr0, lrg = row_groups[-1]
    if tail_split and lrg > tail_split:
        tail_rows += [(lr0, lrg - tail_split), (lr0 + lrg - tail_split, tail_split)]
    else:
        tail_rows.append((lr0, lrg))
    st2_tail = [((1 + r0) * Wp, rg * Wp, r0, rg) for r0, rg in tail_rows]
    # image 0 is prepped in 7-row pieces (NP1 of them) with conv1 stretches
    # aligned so stretch s only reads input rows loaded by pieces <= s. The
    # +Wp+1 shifted window spills one byte into the row after r0+rg, so
    # reserve one extra row per boundary.
    GR1 = GR  # rows per image-0 prep piece
    NP1 = H // GR1
    rg1 = []
    r = 0
    for s in range(NP1):
        hi = GR1 * (s + 1) - 2
        if s == NP1 - 1:
            hi = H
        rg1.append((r, hi - r))
        r = hi
    st1_first = [((1 + r0) * Wp, rg * Wp, r0, rg) for r0, rg in rg1]
    st1_rest = st2
    PS_COLS = psum_stretch

    nc = bacc.Bacc("TRN2", target_bir_lowering=False, debug=False)

    x_h = nc.dram_tensor("x", [B, C, S], F32, kind="ExternalInput")
    cb_h = nc.dram_tensor("cb", [KP, CONST_B], U8, kind="ExternalInput")
    y_h = nc.dram_tensor("y", [B, NCHUNK, KP, PO], F32, kind="ExternalOutput")

    def dram_ap(handle, offset, dims):
        return bass.AP(
            tensor=handle.ap().tensor, offset=offset, ap=[list(d) for d in dims]
        )

    with tile.TileContext(nc) as tc:
        from contextlib import ExitStack

        with ExitStack() as ctx:
            consts = ctx.enter_context(tc.tile_pool(name="consts", bufs=1))
            xnat_p = ctx.enter_context(tc.tile_pool(name="xnat", bufs=1))
            xsT_p = ctx.enter_context(tc.tile_pool(name="xsT", bufs=2))
            hsT_p = ctx.enter_context(tc.tile_pool(name="hsT", bufs=2))
            pr_p = ctx.enter_context(tc.tile_pool(name="prp", bufs=2))
            po_p = ctx.enter_context(tc.tile_pool(name="pop", bufs=2))
            convp = ctx.enter_context(
                tc.tile_pool(name="convp", bufs=conv_bufs, space="PSUM")
            )

            # --- packed constants (three DMAs: w1-j0+nt1, w1-j1, rest)
            cb = consts.tile([KP, CONST_B], U8)
            w1j = [
                cb[:, W1J0_OFF : W1J0_OFF + 2304].bitcast(FP8).rearrange(
                    "p (t k m) -> p t k m", t=9, k=2
                ),
                cb[:, W1J1_OFF : W1J1_OFF + 2304].bitcast(FP8).rearrange(
                    "p (t k m) -> p t k m", t=9, k=2
                ),
            ]
            w2sb = cb[:, W2_OFF : W2_OFF + 4608].bitcast(FP8).rearrange(
                "p (t j k m) -> p t j k m", t=9, j=NCHUNK, k=2
            )
            w1v = lambda j, t: w1j[j][:, t]
            w2v = lambda j, t: w2sb[:, t, j]
            nt1sb = cb[:, NT1_OFF : NT1_OFF + 8].bitcast(F32)
            s2sb = cb[:, S2_OFF : S2_OFF + 8].bitcast(F32)
            b2sb = cb[:, B2_OFF : B2_OFF + 8].bitcast(F32)

            # --- preload the ACT piecewise-poly table (Sign) with a tiny
            # dependency-free activation so the 1.3us table load is off the
            # critical prep chain
            dummy = consts.tile([1, 4], F32)
            nc.vector.memset(dummy, 0.0)
            nc.scalar.activation(dummy, dummy, Act.Sign, bias=0.0, scale=1.0)

            from concourse import masks

            id8sb = consts.tile([KP, KP], BF16)
            masks.make_identity(nc, id8sb)

            # --- PE p-state warmup: dependency-free junk transposes keep the
            # tensor engine busy from t~0 so the ramp is spent before real
            # conv matmuls arrive. The junk lives in a convp rotation buffer
            # (conv matmuls start=True overwrite it later).
            if warm_mm:
                warm = convp.tile([KP, KP], BF16, tag="cv", name="warm")
                for _ in range(warm_mm):
                    nc.tensor.transpose(warm, id8sb, id8sb)

            # --- loads, issued in priority order (DMA transfers serialize)
            xn = {}
            for img in range(B):
                xn[img] = xnat_p.tile(
                    [KP, NCHUNK, S], F32, tag=f"xn{img}", name=f"xn{img}"
                )

            def load_x_span(img, s0, s1):
                nc.sync.dma_start(
                    out=xn[img][:, :, s0:s1],
                    in_=dram_ap(
                        x_h,
                        img * C * S + s0,
                        [[S, KP], [KP * S, NCHUNK], [1, s1 - s0]],
                    ),
                )

            Q = GR * W   # spatial elems per steady-state prep quarter
            Q1 = GR1 * W  # spatial elems per image-0 piece
            load_x_span(0, 0, Q1)
            nc.sync.dma_start(out=cb[:, :CBA_B], in_=cb_h.ap()[:, :CBA_B])
            load_x_span(0, Q1, 2 * Q1)
            nc.sync.dma_start(out=cb[:, CBA_B:CBB_B], in_=cb_h.ap()[:, CBA_B:CBB_B])
            for k in range(2, NP1):
                load_x_span(0, k * Q1, (k + 1) * Q1)
            nc.sync.dma_start(out=cb[:, CBB_B:], in_=cb_h.ap()[:, CBB_B:])
            for img in range(1, B):
                load_x_span(img, 0, S // 2)
                load_x_span(img, S // 2, S)

            # --- helpers
            def border_memsets(buf):
                # rows 0 and H+1, left/right pads, and border cols {0, W+1} of
                # rows 1..H; on GPSIMD so the vector engines stay free.
                nc.gpsimd.memset(buf[:, :, 0 : DOFF + Wp], 0.0)
                nc.gpsimd.memset(buf[:, :, DOFF + (H + 1) * Wp : S_chunk], 0.0)
                rows = buf[:, :, DOFF + Wp : DOFF + (H + 1) * Wp].rearrange(
                    "p j (r w) -> p j r w", w=Wp
                )
                nc.gpsimd.memset(rows[:, :, :, 0 :: (W + 1)], 0.0)

            xsT_tiles = {}

            def prep_span(img, lo, hi):
                # fused sign+scatter of rows [lo, hi): fp32 -> fp8 +-0.5
                # written straight into the padded conv layout; j0 on DVE,
                # j1 on GPSIMD so the two planes run in parallel
                if lo == 0:
                    xsT_tiles[img] = xsT_p.tile(
                        [KP, NCHUNK, S_chunk], FP8, tag="xsT", name=f"xsT{img}"
                    )
                    border_memsets(xsT_tiles[img])
                xsT = xsT_tiles[img]
                a0 = DOFF + (1 + lo) * Wp
                for j in range(NCHUNK):
                    src = xn[img][:, j, lo * W : hi * W].rearrange(
                        "p (r w) -> p r w", w=W
                    )
                    dst = xsT[:, j, a0 : a0 + (hi - lo) * Wp].rearrange(
                        "p (r w) -> p r w", w=Wp
                    )[:, :, 1 : 1 + W]
                    eng = nc.vector if j == 0 else nc.gpsimd
                    eng.tensor_scalar(
                        dst, src, 0.0, 0.5, Alu.is_ge, Alu.subtract
                    )

            def prep_group(img, g):
                prep_span(img, g * GR, (g + 1) * GR)

            def conv_stretch(inbuf, wv, st, si, j, psum_cb, nm):
                cs, cn, r0, rg = st
                ps = convp.tile([KP, PS_COLS], F32, tag="cv", name=f"cv{nm}{si}{j}")
                if exact_rows:
                    # per-row 56-col matmuls (skip the 2 pad cols per row),
                    # row-outer/tap-inner so each row's PSUM accumulation
                    # group opens and closes before the next row touches the
                    # same 2KB bank. Rows sit at a 64-col pitch so no matmul
                    # output straddles a bank.
                    for r in range(rg):
                        for t in range(9):
                            dy, dx = t // 3, t % 3
                            a = DOFF + (r0 + r + dy) * Wp + dx
                            nc.tensor.matmul(
                                ps[:, r * 64 : r * 64 + W],
                                wv(j, t),
                                inbuf[:, :, a : a + W],
                                start=(t == 0),
                                stop=(t == 8),
                                perf_mode=DR,
                            )
                else:
                    for t in range(9):
                        dy, dx = t // 3, t % 3
                        lhsT = wv(j, t)
                        off = (dy - 1) * Wp + (dx - 1)
                        for c0 in range(0, cn, 512):
                            n = min(512, cn - c0)
                            a = DOFF + cs + off + c0
                            nc.tensor.matmul(
                                ps[:, c0 : c0 + n],
                                lhsT,
                                inbuf[:, :, a : a + n],
                                start=(t == 0),
                                stop=(t == 8),
                                perf_mode=DR,
                            )
                psum_cb(si, j, ps, st)

            hsT_tiles = {}

            def conv1_stretch(img, si):
                sts1 = st1_first if img == 0 else st1_rest
                st = sts1[si]
                if si == 0:
                    hsT_tiles[img] = hsT_p.tile(
                        [KP, NCHUNK, S_chunk], FP8, tag="hsT", name=f"hsT{img}"
                    )
                    border_memsets(hsT_tiles[img])
                hsT = hsT_tiles[img]

                def bnsign(si_, j, ps, st_):
                    cs, cn, r0_, rg_ = st_
                    dstv = hsT[:, j, DOFF + cs : DOFF + cs + cn].rearrange(
                        "p (r w) -> p r w", w=Wp
                    )[:, :, 1 : 1 + W]
                    if exact_rows:
                        srcv = ps[:, : rg_ * 64].rearrange("p (r w) -> p r w", w=64)[
                            :, :, :W
                        ]
                    else:
                        srcv = ps[:, :cn].rearrange("p (r w) -> p r w", w=Wp)[
                            :, :, 1 : 1 + W
                        ]
                    nc.scalar.activation(
                        dstv, srcv, Act.Sign, bias=nt1sb[:, j : j + 1], scale=1.0
                    )

                for j in range(NCHUNK):
                    conv_stretch(xsT_tiles[img], w1v, st, si, j, bnsign, f"a{img}")
                if si == len(sts1) - 1:
                    xsT_tiles.pop(img)

            pr_tiles = {}
            pooled_tiles = {}

            def conv2_stretch(img, si):
                sts = st2_tail if img == B - 1 else st2
                st = sts[si]
                if si == 0:
                    pr_tiles[img] = [
                        pr_p.tile([KP, H // 2, W], F32, tag="pr", name=f"pr{img}{j}")
                        for j in range(NCHUNK)
                    ]
                    pooled_tiles[img] = [
                        po_p.tile([KP, PO], F32, tag="pooled", name=f"pl{img}{j}")
                        for j in range(NCHUNK)
                    ]

                def pool_cb(si_, j, ps, st_):
                    cs, cn, r0, rg = st_
                    if exact_rows:
                        rows = ps[:, : rg * 64].rearrange("p (q t) -> p q t", t=128)
                        in0 = rows[:, :, 0:W]
                        in1 = rows[:, :, 64 : 64 + W]
                    else:
                        rows = ps[:, : rg * Wp].rearrange("p (q t) -> p q t", t=2 * Wp)
                        in0 = rows[:, :, 1 : 1 + W]
                        in1 = rows[:, :, Wp + 1 : Wp + 1 + W]
                    q0, q1 = r0 // 2, (r0 + rg) // 2
                    q = rg // 2
                    # TensorTensor may read only one input from PSUM: stage
                    # the even rows into SBUF, then max against the PSUM odd
                    # rows. Steady state splits the copy to ACT for engine
                    # parallelism; the last image's short tail chains run
                    # entirely on DVE to avoid cross-engine sem hops.
                    prA = pr_p.tile(
                        [KP, max_rows // 2, W], F32, tag="prA", bufs=4,
                        name=f"prA{img}{si_}{j}",
                    )
                    nc.scalar.copy(prA[:, :q, :], in0)
                    nc.vector.tensor_max(
                        pr_tiles[img][j][:, q0:q1, :], prA[:, :q, :], in1
                    )
                    prs = pr_tiles[img][j][:, q0:q1, :].rearrange("p q w -> p (q w)")
                    pv = pooled_tiles[img][j].rearrange("p (q w) -> p q w", w=WH)[
                        :, q0:q1, :
                    ]
                    nc.vector.tensor_max(pv, prs[:, 0::2], prs[:, 1::2])
                    nc.vector.tensor_scalar(
                        pv, pv, s2sb[:, j : j + 1], b2sb[:, j : j + 1],
                        Alu.mult, Alu.add,
                    )
                    # stores: whole channel-chunk per image, but per-stretch
                    # for the last image so the tail ships immediately
                    if img == B - 1:
                        nc.sync.dma_start(
                            out=dram_ap(
                                y_h,
                                (img * NCHUNK + j) * KP * PO + q0 * WH,
                                [[PO, KP], [1, (q1 - q0) * WH]],
                            ),
                            in_=pooled_tiles[img][j][:, q0 * WH : q1 * WH],
                        )
                    elif si_ == len(sts) - 1:
                        nc.sync.dma_start(
                            out=dram_ap(
                                y_h,
                                (img * NCHUNK + j) * KP * PO,
                                [[PO, KP], [1, PO]],
                            ),
                            in_=pooled_tiles[img][j],
                        )

                for j in range(NCHUNK):
                    conv_stretch(hsT_tiles[img], w2v, st, si, j, pool_cb, f"b{img}")
                if si == len(sts) - 1:
                    hsT_tiles.pop(img)

            # --- emission ---
            # image 0: prep each 7-row piece right before the conv1 stretch
            # that needs it. The conv rhs spans both channel planes as one
            # interval hull, so any copy emitted before a stretch becomes a
            # dependency of it — never emit a copy ahead of an earlier
            # stretch.
            for k in range(NP1):
                prep_span(0, k * GR1, (k + 1) * GR1)
                conv1_stretch(0, k)
            for img in range(B):
                if img > 0:
                    for si in range(len(st2)):
                        conv1_stretch(img, si)
                        if img + 1 < B:
                            prep_group(img + 1, si)
                for si in range(len(st2_tail if img == B - 1 else st2)):
                    if img == 0 and B > 1 and si < NQ:
                        prep_group(1, si)
                    conv2_stretch(img, si)

    nc.compile()
    return nc


# ---------------------------------------------------------------------------
# host-side constant prep
# ---------------------------------------------------------------------------


def _prep_consts(w1, beta1, mean1, var1, w2, beta2, mean2, var2):
    import jax
    import jax.numpy as jnp
    from jax import lax
    from concourse import mybir

    fp8np = mybir.dt.np(mybir.dt.float8e4)

    def prep_w(w, j_major=False):
        ws = np.where(np.asarray(w) >= 0, np.float32(1.0), np.float32(-1.0))
        # [3,3,ci,co] -> [p, (j,) tap, ktile, m]; ci = ktile*128+p, co = j*128+m
        wr = ws.reshape(9, 2, KP, NCHUNK, KP)
        wr = wr.transpose((2, 3, 0, 1, 4) if j_major else (2, 0, 3, 1, 4))
        return np.ascontiguousarray(wr).astype(fp8np)

    w1p, w2p = prep_w(w1, j_major=True), prep_w(w2)

    cpu = jax.devices("cpu")[0]
    MAXH = 9 * C
    with jax.default_device(cpu):
        hs = jnp.arange(-MAXH, MAXH + 1, dtype=jnp.float32)
        bn1 = (hs[:, None] - jnp.asarray(mean1)[None, :]) * lax.rsqrt(
            jnp.asarray(var1) + 1e-3
        )[None, :] + jnp.asarray(beta1)[None, :]
        nonneg = np.asarray(bn1 >= 0)
        r2 = np.asarray(lax.rsqrt(jnp.asarray(var2) + 1e-3))

    assert (np.diff(nonneg.astype(np.int8), axis=0) >= 0).all(), "bn1 not monotone"
    kc = np.where(nonneg.any(0), nonneg.argmax(0), 2 * MAXH + 1) - MAXH
    # device psum holds h/2 (x=+-0.5, w=+-1): sign flips at (kc-0.5)/2
    nt1 = (-(kc.astype(np.float64) - 0.5) / 2.0).astype(np.float32)

    s2 = r2.astype(np.float32)
    b2 = (
        np.asarray(beta2, np.float64)
        - np.asarray(mean2, np.float64) * s2.astype(np.float64)
    ).astype(np.float32)

    def to_pj(a):  # [256] -> [128, 2] with c = j*128+p
        return np.ascontiguousarray(a.reshape(NCHUNK, KP).T).astype(np.float32)

    # pack everything into one [128, CONST_B] uint8 image
    cbuf = np.zeros((KP, CONST_B), dtype=np.uint8)

    def put(off, arr):
        by = np.ascontiguousarray(arr).reshape(KP, -1).view(np.uint8)
        cbuf[:, off : off + by.shape[1]] = by

    put(W1J0_OFF, w1p[:, 0])
    put(W1J1_OFF, w1p[:, 1])
    put(W2_OFF, w2p)
    put(NT1_OFF, to_pj(nt1))
    put(S2_OFF, to_pj(s2))
    put(B2_OFF, to_pj(b2))
    return {"cb": cbuf}


# ---------------------------------------------------------------------------
# entry point
# ---------------------------------------------------------------------------

_cached = {}


def _run(inputs, trace=False):
    from concourse import bass_utils

    x = np.asarray(inputs["x"], dtype=np.float32)
    Bt, H, W, _ = x.shape  # 32, 56, 56, 256
    Bc = Bt // N_CORES

    consts = _prep_consts(
        inputs["w1"], inputs["beta1"], inputs["mean1"], inputs["var1"],
        inputs["w2"], inputs["beta2"], inputs["mean2"], inputs["var2"],
    )

    key = (Bc, H, W)
    if key not in _cached:
        _cached[key] = build_program(Bc, H, W)
    nc = _cached[key]

    # channel-major marshaling: [Bt, H*W, C] -> per-core [Bc, C, H*W]
    xcm = np.ascontiguousarray(
        x.reshape(Bt, H * W, C).transpose(0, 2, 1)
    )

    in_maps = []
    for c in range(N_CORES):
        m = dict(consts)
        m["x"] = xcm[c * Bc : (c + 1) * Bc]
        in_maps.append(m)

    res = bass_utils.run_bass_kernel_spmd(
        nc, in_maps, core_ids=list(range(N_CORES)), trace=trace
    )
    PO = (H // 2) * (W // 2)
    # y comes back channel-major [Bc, 2, 128, PO]; restore NHWC
    y = np.concatenate(
        [r["y"].reshape(Bc, C, PO).transpose(0, 2, 1) for r in res.results], axis=0
    )
    y = np.ascontiguousarray(y.reshape(Bt, H // 2, W // 2, C)).astype(np.float32)
    return y, res


def kernel(**inputs):
    y, _ = _run(inputs, trace=False)
    return y


# revision 49
# speedup vs baseline: 1.2893x; 1.0042x over previous
"""Trainium2 Bass kernel for a BinaryNet conv block.

Pipeline (per core, data-parallel over batch):
  sign(x) -> conv3x3(sign(w1)) -> BN1 -> sign -> conv3x3(sign(w2))
          -> maxpool2x2 -> BN2

Implementation notes:
  - Activations are +-0.5, weights +-1.0 in fp8e4 (exactly representable);
    convs run as 9 shifted-window matmuls with DoubleRow perf mode (K=256
    contraction per instruction), accumulating exactly into fp32 PSUM.
  - BN1+sign is fused into one ScalarE Sign activation against a
    host-precomputed per-channel threshold. Conv outputs are exact
    integers, so an integer cutoff k_c reproduces the reference's fp32
    sign decisions bit-exactly.
  - The host marshals x to channel-major [C, H*W] per image and reads y
    back channel-major [2, 128, PO]; the device never transposes. The PE
    therefore runs conv matmuls only, fed by DVE sign + ACT/DVE scatter
    copies into the zero-bordered padded layout.
  - Spatial layout is channel-major [ci, y*(W+2)+x] with a zero border so
    the 9 taps are just constant AP offsets.
  - The bass2jax/pseudo-DMA path allows only ONE sync wait per DMA; every
    DMA destination is a fresh tile (or a disjoint slice of one), so no
    DMA ever needs more than one semaphore wait. All loads are issued
    up-front in priority order (DMA transfers serialize), stores as
    produced.
  - A short burst of junk transposes warms the PE p-state ramp so the
    first real conv matmuls run at full clock.
"""

import os
import numpy as np

os.environ.setdefault("MYCRO_LOCAL_CACHE", "1")

N_CORES = 8
C = 256
NCHUNK = 2  # channel chunks of 128
KP = 128

# packed consts layout (bytes per partition); w1 split by output-channel
# chunk so the first conv can start as soon as the j0 half lands
W1J0_OFF = 0        # fp8 [9,2,128] -> 2304 B
NT1_OFF = 2304      # f32 [2] -> 8 B
CBA_B = 2312        # first consts DMA covers [0, CBA_B)
W1J1_OFF = 2312     # fp8 -> 2304 B
CBB_B = 4616        # second consts DMA covers [CBA_B, CBB_B)
W2_OFF = 4616       # fp8 [9,2,2,128] -> 4608 B
S2_OFF = 9224       # f32 [2]
B2_OFF = 9232       # f32 [2]
CONST_B = 9240


def build_program(B, H, W, psum_stretch=1024, conv_bufs=4, warm_mm=46, tail_split=0, exact_rows=True):
    """Build the per-core Bass program. B images of HxWxC per core."""
    import concourse.bass as bass
    import concourse.bacc as bacc
    import concourse.tile as tile
    from concourse import mybir

    F32 = mybir.dt.float32
    FP8 = mybir.dt.float8e4
    BF16 = mybir.dt.bfloat16
    U8 = mybir.dt.uint8
    DR = mybir.MatmulPerfMode.DoubleRow
    Alu = mybir.AluOpType
    Act = mybir.ActivationFunctionType

    Hp, Wp = H + 2, W + 2
    S_pad = Hp * Wp
    S = H * W
    DOFF = 32  # left zero pad inside each channel-chunk row buffer
    S_chunk = ((S_pad + DOFF + 32 + 15) // 16) * 16  # right pad >= 32
    NQ = 4  # prep groups (and img-0 load quarters) per image
    GR = H // NQ  # rows per prep group
    assert H % NQ == 0
    PO = (H // 2) * (W // 2)
    WH = W // 2

    # conv2 row groups (pool-pair aligned)
    max_rows = (psum_stretch // Wp) // 2 * 2
    row_groups = []
    r = 0
    while r < H:
        g = min(max_rows, H - r)
        row_groups.append((r, g))
        r += g
    st2 = [((1 + r0) * Wp, rg * Wp, r0, rg) for r0, rg in row_groups]
    # last image: split the final row group so the pool/store chain after
    # the very last matmul is as short as possible
    tail_rows = list(row_groups[:-1])
    lr0, lrg = row_groups[-1]
    if tail_split and lrg > tail_split:
        tail_rows += [(lr0, lrg - tail_split), (lr0 + lrg - tail_split, tail_split)]
    else:
        tail_rows.append((lr0, lrg))
    st2_tail = [((1 + r0) * Wp, rg * Wp, r0, rg) for r0, rg in tail_rows]
    # image 0 is prepped in 7-row pieces (NP1 of them) with conv1 stretches
    # aligned so stretch s only reads input rows loaded by pieces <= s. The
    # +Wp+1 shifted window spills one byte into the row after r0+rg, so
    # reserve one extra row per boundary.
    # pieces: two 7-row halves of the first quarter, then whole quarters.
    # stretch s may read up to one row past its end, so each stretch stops
    # two rows short of its piece's cumulative coverage.
    GRH = GR // 2
    p0 = [(0, GRH), (GRH, GRH)] + [(GR * k, GR) for k in range(1, NQ)]
    rg1 = []
    r = 0
    cum = 0
    for i, (plo, pn) in enumerate(p0):
        cum += pn
        hi = H if i == len(p0) - 1 else cum - 2
        rg1.append((r, hi - r))
        r = hi
    NP1 = len(p0)
    st1_first = [((1 + r0) * Wp, rg * Wp, r0, rg) for r0, rg in rg1]
    st1_rest = st2
    PS_COLS = psum_stretch

    nc = bacc.Bacc("TRN2", target_bir_lowering=False, debug=False)

    x_h = nc.dram_tensor("x", [B, C, S], F32, kind="ExternalInput")
    cb_h = nc.dram_tensor("cb", [KP, CONST_B], U8, kind="ExternalInput")
    y_h = nc.dram_tensor("y", [B, NCHUNK, KP, PO], F32, kind="ExternalOutput")

    def dram_ap(handle, offset, dims):
        return bass.AP(
            tensor=handle.ap().tensor, offset=offset, ap=[list(d) for d in dims]
        )

    with tile.TileContext(nc) as tc:
        from contextlib import ExitStack

        with ExitStack() as ctx:
            consts = ctx.enter_context(tc.tile_pool(name="consts", bufs=1))
            xnat_p = ctx.enter_context(tc.tile_pool(name="xnat", bufs=1))
            xsT_p = ctx.enter_context(tc.tile_pool(name="xsT", bufs=2))
            hsT_p = ctx.enter_context(tc.tile_pool(name="hsT", bufs=2))
            pr_p = ctx.enter_context(tc.tile_pool(name="prp", bufs=2))
            po_p = ctx.enter_context(tc.tile_pool(name="pop", bufs=2))
            convp = ctx.enter_context(
                tc.tile_pool(name="convp", bufs=conv_bufs, space="PSUM")
            )

            # --- packed constants (three DMAs: w1-j0+nt1, w1-j1, rest)
            cb = consts.tile([KP, CONST_B], U8)
            w1j = [
                cb[:, W1J0_OFF : W1J0_OFF + 2304].bitcast(FP8).rearrange(
                    "p (t k m) -> p t k m", t=9, k=2
                ),
                cb[:, W1J1_OFF : W1J1_OFF + 2304].bitcast(FP8).rearrange(
                    "p (t k m) -> p t k m", t=9, k=2
                ),
            ]
            w2sb = cb[:, W2_OFF : W2_OFF + 4608].bitcast(FP8).rearrange(
                "p (t j k m) -> p t j k m", t=9, j=NCHUNK, k=2
            )
            w1v = lambda j, t: w1j[j][:, t]
            w2v = lambda j, t: w2sb[:, t, j]
            nt1sb = cb[:, NT1_OFF : NT1_OFF + 8].bitcast(F32)
            s2sb = cb[:, S2_OFF : S2_OFF + 8].bitcast(F32)
            b2sb = cb[:, B2_OFF : B2_OFF + 8].bitcast(F32)

            # --- preload the ACT piecewise-poly table (Sign) with a tiny
            # dependency-free activation so the 1.3us table load is off the
            # critical prep chain
            dummy = consts.tile([1, 4], F32)
            nc.vector.memset(dummy, 0.0)
            nc.scalar.activation(dummy, dummy, Act.Sign, bias=0.0, scale=1.0)

            from concourse import masks

            id8sb = consts.tile([KP, KP], BF16)
            masks.make_identity(nc, id8sb)

            # --- PE p-state warmup: dependency-free junk transposes keep the
            # tensor engine busy from t~0 so the ramp is spent before real
            # conv matmuls arrive. The junk lives in a convp rotation buffer
            # (conv matmuls start=True overwrite it later).
            if warm_mm:
                warm = convp.tile([KP, KP], BF16, tag="cv", name="warm")
                for _ in range(warm_mm):
                    nc.tensor.transpose(warm, id8sb, id8sb)

            # --- loads, issued in priority order (DMA transfers serialize)
            xn = {}
            for img in range(B):
                xn[img] = xnat_p.tile(
                    [KP, NCHUNK, S], F32, tag=f"xn{img}", name=f"xn{img}"
                )

            def load_x_span(img, s0, s1):
                nc.sync.dma_start(
                    out=xn[img][:, :, s0:s1],
                    in_=dram_ap(
                        x_h,
                        img * C * S + s0,
                        [[S, KP], [KP * S, NCHUNK], [1, s1 - s0]],
                    ),
                )

            Q = GR * W   # spatial elems per steady-state prep quarter
            load_x_span(0, 0, GRH * W)
            nc.sync.dma_start(out=cb[:, :CBA_B], in_=cb_h.ap()[:, :CBA_B])
            load_x_span(0, GRH * W, GR * W)
            nc.sync.dma_start(out=cb[:, CBA_B:CBB_B], in_=cb_h.ap()[:, CBA_B:CBB_B])
            for k in range(1, NQ):
                load_x_span(0, k * Q, (k + 1) * Q)
            nc.sync.dma_start(out=cb[:, CBB_B:], in_=cb_h.ap()[:, CBB_B:])
            for img in range(1, B):
                load_x_span(img, 0, S // 2)
                load_x_span(img, S // 2, S)

            # --- helpers
            def border_memsets(buf):
                # rows 0 and H+1, left/right pads, and border cols {0, W+1} of
                # rows 1..H; on GPSIMD so the vector engines stay free.
                nc.gpsimd.memset(buf[:, :, 0 : DOFF + Wp], 0.0)
                nc.gpsimd.memset(buf[:, :, DOFF + (H + 1) * Wp : S_chunk], 0.0)
                rows = buf[:, :, DOFF + Wp : DOFF + (H + 1) * Wp].rearrange(
                    "p j (r w) -> p j r w", w=Wp
                )
                nc.gpsimd.memset(rows[:, :, :, 0 :: (W + 1)], 0.0)

            xsT_tiles = {}

            def prep_span(img, lo, hi):
                # fused sign+scatter of rows [lo, hi): fp32 -> fp8 +-0.5
                # written straight into the padded conv layout; j0 on DVE,
                # j1 on GPSIMD so the two planes run in parallel
                if lo == 0:
                    xsT_tiles[img] = xsT_p.tile(
                        [KP, NCHUNK, S_chunk], FP8, tag="xsT", name=f"xsT{img}"
                    )
                    border_memsets(xsT_tiles[img])
                xsT = xsT_tiles[img]
                a0 = DOFF + (1 + lo) * Wp
                for j in range(NCHUNK):
                    src = xn[img][:, j, lo * W : hi * W].rearrange(
                        "p (r w) -> p r w", w=W
                    )
                    dst = xsT[:, j, a0 : a0 + (hi - lo) * Wp].rearrange(
                        "p (r w) -> p r w", w=Wp
                    )[:, :, 1 : 1 + W]
                    eng = nc.vector if j == 0 else nc.gpsimd
                    eng.tensor_scalar(
                        dst, src, 0.0, 0.5, Alu.is_ge, Alu.subtract
                    )

            def prep_group(img, g):
                prep_span(img, g * GR, (g + 1) * GR)

            def conv_stretch(inbuf, wv, st, si, j, psum_cb, nm):
                cs, cn, r0, rg = st
                ps = convp.tile([KP, PS_COLS], F32, tag="cv", name=f"cv{nm}{si}{j}")
                if exact_rows:
                    # per-row 56-col matmuls (skip the 2 pad cols per row),
                    # row-outer/tap-inner so each row's PSUM accumulation
                    # group opens and closes before the next row touches the
                    # same 2KB bank. Rows sit at a 64-col pitch so no matmul
                    # output straddles a bank.
                    for r in range(rg):
                        for t in range(9):
                            dy, dx = t // 3, t % 3
                            a = DOFF + (r0 + r + dy) * Wp + dx
                            nc.tensor.matmul(
                                ps[:, r * 64 : r * 64 + W],
                                wv(j, t),
                                inbuf[:, :, a : a + W],
                                start=(t == 0),
                                stop=(t == 8),
                                perf_mode=DR,
                            )
                else:
                    for t in range(9):
                        dy, dx = t // 3, t % 3
                        lhsT = wv(j, t)
                        off = (dy - 1) * Wp + (dx - 1)
                        for c0 in range(0, cn, 512):
                            n = min(512, cn - c0)
                            a = DOFF + cs + off + c0
                            nc.tensor.matmul(
                                ps[:, c0 : c0 + n],
                                lhsT,
                                inbuf[:, :, a : a + n],
                                start=(t == 0),
                                stop=(t == 8),
                                perf_mode=DR,
                            )
                psum_cb(si, j, ps, st)

            hsT_tiles = {}

            def conv1_stretch(img, si):
                sts1 = st1_first if img == 0 else st1_rest
                st = sts1[si]
                if si == 0:
                    hsT_tiles[img] = hsT_p.tile(
                        [KP, NCHUNK, S_chunk], FP8, tag="hsT", name=f"hsT{img}"
                    )
                    border_memsets(hsT_tiles[img])
                hsT = hsT_tiles[img]

                def bnsign(si_, j, ps, st_):
                    cs, cn, r0_, rg_ = st_
                    dstv = hsT[:, j, DOFF + cs : DOFF + cs + cn].rearrange(
                        "p (r w) -> p r w", w=Wp
                    )[:, :, 1 : 1 + W]
                    if exact_rows:
                        srcv = ps[:, : rg_ * 64].rearrange("p (r w) -> p r w", w=64)[
                            :, :, :W
                        ]
                    else:
                        srcv = ps[:, :cn].rearrange("p (r w) -> p r w", w=Wp)[
                            :, :, 1 : 1 + W
                        ]
                    nc.scalar.activation(
                        dstv, srcv, Act.Sign, bias=nt1sb[:, j : j + 1], scale=1.0
                    )

                for j in range(NCHUNK):
                    conv_stretch(xsT_tiles[img], w1v, st, si, j, bnsign, f"a{img}")
                if si == len(sts1) - 1:
                    xsT_tiles.pop(img)

            pr_tiles = {}
            pooled_tiles = {}

            def conv2_stretch(img, si):
                sts = st2_tail if img == B - 1 else st2
                st = sts[si]
                if si == 0:
                    pr_tiles[img] = [
                        pr_p.tile([KP, H // 2, W], F32, tag="pr", name=f"pr{img}{j}")
                        for j in range(NCHUNK)
                    ]
                    pooled_tiles[img] = [
                        po_p.tile([KP, PO], F32, tag="pooled", name=f"pl{img}{j}")
                        for j in range(NCHUNK)
                    ]

                def pool_cb(si_, j, ps, st_):
                    cs, cn, r0, rg = st_
                    if exact_rows:
                        rows = ps[:, : rg * 64].rearrange("p (q t) -> p q t", t=128)
                        in0 = rows[:, :, 0:W]
                        in1 = rows[:, :, 64 : 64 + W]
                    else:
                        rows = ps[:, : rg * Wp].rearrange("p (q t) -> p q t", t=2 * Wp)
                        in0 = rows[:, :, 1 : 1 + W]
                        in1 = rows[:, :, Wp + 1 : Wp + 1 + W]
                    q0, q1 = r0 // 2, (r0 + rg) // 2
                    q = rg // 2
                    # TensorTensor may read only one input from PSUM: stage
                    # the even rows into SBUF, then max against the PSUM odd
                    # rows. Steady state splits the copy to ACT for engine
                    # parallelism; the last image's short tail chains run
                    # entirely on DVE to avoid cross-engine sem hops.
                    prA = pr_p.tile(
                        [KP, max_rows // 2, W], F32, tag="prA", bufs=4,
                        name=f"prA{img}{si_}{j}",
                    )
                    nc.scalar.copy(prA[:, :q, :], in0)
                    nc.vector.tensor_max(
                        pr_tiles[img][j][:, q0:q1, :], prA[:, :q, :], in1
                    )
                    prs = pr_tiles[img][j][:, q0:q1, :].rearrange("p q w -> p (q w)")
                    pv = pooled_tiles[img][j].rearrange("p (q w) -> p q w", w=WH)[
                        :, q0:q1, :
                    ]
                    nc.vector.tensor_max(pv, prs[:, 0::2], prs[:, 1::2])
                    nc.vector.tensor_scalar(
                        pv, pv, s2sb[:, j : j + 1], b2sb[:, j : j + 1],
                        Alu.mult, Alu.add,
                    )
                    # stores: whole channel-chunk per image, but per-stretch
                    # for the last image so the tail ships immediately
                    if img == B - 1:
                        nc.sync.dma_start(
                            out=dram_ap(
                                y_h,
                                (img * NCHUNK + j) * KP * PO + q0 * WH,
                                [[PO, KP], [1, (q1 - q0) * WH]],
                            ),
                            in_=pooled_tiles[img][j][:, q0 * WH : q1 * WH],
                        )
                    elif si_ == len(sts) - 1:
                        nc.sync.dma_start(
                            out=dram_ap(
                                y_h,
                                (img * NCHUNK + j) * KP * PO,
                                [[PO, KP], [1, PO]],
                            ),
                            in_=pooled_tiles[img][j],
                        )

                for j in range(NCHUNK):
                    conv_stretch(hsT_tiles[img], w2v, st, si, j, pool_cb, f"b{img}")
                if si == len(sts) - 1:
                    hsT_tiles.pop(img)

            # --- emission ---
            # image 0: prep each 7-row piece right before the conv1 stretch
            # that needs it. The conv rhs spans both channel planes as one
            # interval hull, so any copy emitted before a stretch becomes a
            # dependency of it — never emit a copy ahead of an earlier
            # stretch.
            for k in range(NP1):
                plo, pn = p0[k]
                prep_span(0, plo, plo + pn)
                conv1_stretch(0, k)
            for img in range(B):
                if img > 0:
                    for si in range(len(st2)):
                        conv1_stretch(img, si)
                        if img + 1 < B:
                            prep_group(img + 1, si)
                for si in range(len(st2_tail if img == B - 1 else st2)):
                    if img == 0 and B > 1 and si < NQ:
                        prep_group(1, si)
                    conv2_stretch(img, si)

    nc.compile()
    return nc


# ---------------------------------------------------------------------------
# host-side constant prep
# ---------------------------------------------------------------------------


def _prep_consts(w1, beta1, mean1, var1, w2, beta2, mean2, var2):
    import jax
    import jax.numpy as jnp
    from jax import lax
    from concourse import mybir

    fp8np = mybir.dt.np(mybir.dt.float8e4)

    def prep_w(w, j_major=False):
        ws = np.where(np.asarray(w) >= 0, np.float32(1.0), np.float32(-1.0))
        # [3,3,ci,co] -> [p, (j,) tap, ktile, m]; ci = ktile*128+p, co = j*128+m
        wr = ws.reshape(9, 2, KP, NCHUNK, KP)
        wr = wr.transpose((2, 3, 0, 1, 4) if j_major else (2, 0, 3, 1, 4))
        return np.ascontiguousarray(wr).astype(fp8np)

    w1p, w2p = prep_w(w1, j_major=True), prep_w(w2)

    cpu = jax.devices("cpu")[0]
    MAXH = 9 * C
    with jax.default_device(cpu):
        hs = jnp.arange(-MAXH, MAXH + 1, dtype=jnp.float32)
        bn1 = (hs[:, None] - jnp.asarray(mean1)[None, :]) * lax.rsqrt(
            jnp.asarray(var1) + 1e-3
        )[None, :] + jnp.asarray(beta1)[None, :]
        nonneg = np.asarray(bn1 >= 0)
        r2 = np.asarray(lax.rsqrt(jnp.asarray(var2) + 1e-3))

    assert (np.diff(nonneg.astype(np.int8), axis=0) >= 0).all(), "bn1 not monotone"
    kc = np.where(nonneg.any(0), nonneg.argmax(0), 2 * MAXH + 1) - MAXH
    # device psum holds h/2 (x=+-0.5, w=+-1): sign flips at (kc-0.5)/2
    nt1 = (-(kc.astype(np.float64) - 0.5) / 2.0).astype(np.float32)

    s2 = r2.astype(np.float32)
    b2 = (
        np.asarray(beta2, np.float64)
        - np.asarray(mean2, np.float64) * s2.astype(np.float64)
    ).astype(np.float32)

    def to_pj(a):  # [256] -> [128, 2] with c = j*128+p
        return np.ascontiguousarray(a.reshape(NCHUNK, KP).T).astype(np.float32)

    # pack everything into one [128, CONST_B] uint8 image
    cbuf = np.zeros((KP, CONST_B), dtype=np.uint8)

    def put(off, arr):
        by = np.ascontiguousarray(arr).reshape(KP, -1).view(np.uint8)
        cbuf[:, off : off + by.shape[1]] = by

    put(W1J0_OFF, w1p[:, 0])
    put(W1J1_OFF, w1p[:, 1])
    put(W2_OFF, w2p)
    put(NT1_OFF, to_pj(nt1))
    put(S2_OFF, to_pj(s2))
    put(B2_OFF, to_pj(b2))
    return {"cb": cbuf}


# ---------------------------------------------------------------------------
# entry point
# ---------------------------------------------------------------------------

_cached = {}


def _run(inputs, trace=False):
    from concourse import bass_utils

    x = np.asarray(inputs["x"], dtype=np.float32)
    Bt, H, W, _ = x.shape  # 32, 56, 56, 256
    Bc = Bt // N_CORES

    consts = _prep_consts(
        inputs["w1"], inputs["beta1"], inputs["mean1"], inputs["var1"],
        inputs["w2"], inputs["beta2"], inputs["mean2"], inputs["var2"],
    )

    key = (Bc, H, W)
    if key not in _cached:
        _cached[key] = build_program(Bc, H, W)
    nc = _cached[key]

    # channel-major marshaling: [Bt, H*W, C] -> per-core [Bc, C, H*W]
    xcm = np.ascontiguousarray(
        x.reshape(Bt, H * W, C).transpose(0, 2, 1)
    )

    in_maps = []
    for c in range(N_CORES):
        m = dict(consts)
        m["x"] = xcm[c * Bc : (c + 1) * Bc]
        in_maps.append(m)

    res = bass_utils.run_bass_kernel_spmd(
        nc, in_maps, core_ids=list(range(N_CORES)), trace=trace
    )
    PO = (H // 2) * (W // 2)
    # y comes back channel-major [Bc, 2, 128, PO]; restore NHWC
    y = np.concatenate(
        [r["y"].reshape(Bc, C, PO).transpose(0, 2, 1) for r in res.results], axis=0
    )
    y = np.ascontiguousarray(y.reshape(Bt, H // 2, W // 2, C)).astype(np.float32)
    return y, res


def kernel(**inputs):
    y, _ = _run(inputs, trace=False)
    return y


# revision 50
# speedup vs baseline: 1.2993x; 1.0077x over previous
"""Trainium2 Bass kernel for a BinaryNet conv block.

Pipeline (per core, data-parallel over batch):
  sign(x) -> conv3x3(sign(w1)) -> BN1 -> sign -> conv3x3(sign(w2))
          -> maxpool2x2 -> BN2

Implementation notes:
  - Activations are +-0.5, weights +-1.0 in fp8e4 (exactly representable);
    convs run as 9 shifted-window matmuls with DoubleRow perf mode (K=256
    contraction per instruction), accumulating exactly into fp32 PSUM.
  - BN1+sign is fused into one ScalarE Sign activation against a
    host-precomputed per-channel threshold. Conv outputs are exact
    integers, so an integer cutoff k_c reproduces the reference's fp32
    sign decisions bit-exactly.
  - The host marshals x to channel-major [C, H*W] per image and reads y
    back channel-major [2, 128, PO]; the device never transposes. The PE
    therefore runs conv matmuls only, fed by DVE sign + ACT/DVE scatter
    copies into the zero-bordered padded layout.
  - Spatial layout is channel-major [ci, y*(W+2)+x] with a zero border so
    the 9 taps are just constant AP offsets.
  - The bass2jax/pseudo-DMA path allows only ONE sync wait per DMA; every
    DMA destination is a fresh tile (or a disjoint slice of one), so no
    DMA ever needs more than one semaphore wait. All loads are issued
    up-front in priority order (DMA transfers serialize), stores as
    produced.
  - A short burst of junk transposes warms the PE p-state ramp so the
    first real conv matmuls run at full clock.
"""

import os
import numpy as np

os.environ.setdefault("MYCRO_LOCAL_CACHE", "1")

N_CORES = 8
C = 256
NCHUNK = 2  # channel chunks of 128
KP = 128

# packed consts layout (bytes per partition); w1 split by output-channel
# chunk so the first conv can start as soon as the j0 half lands
W1J0_OFF = 0        # fp8 [9,2,128] -> 2304 B
NT1_OFF = 2304      # f32 [2] -> 8 B
CBA_B = 2312        # first consts DMA covers [0, CBA_B)
W1J1_OFF = 2312     # fp8 -> 2304 B
CBB_B = 4616        # second consts DMA covers [CBA_B, CBB_B)
W2_OFF = 4616       # fp8 [9,2,2,128] -> 4608 B
S2_OFF = 9224       # f32 [2]
B2_OFF = 9232       # f32 [2]
CONST_B = 9240


def build_program(B, H, W, psum_stretch=1024, conv_bufs=4, warm_mm=46, tail_split=0, exact_rows=True):
    """Build the per-core Bass program. B images of HxWxC per core."""
    import concourse.bass as bass
    import concourse.bacc as bacc
    import concourse.tile as tile
    from concourse import mybir

    F32 = mybir.dt.float32
    FP8 = mybir.dt.float8e4
    BF16 = mybir.dt.bfloat16
    U8 = mybir.dt.uint8
    DR = mybir.MatmulPerfMode.DoubleRow
    Alu = mybir.AluOpType
    Act = mybir.ActivationFunctionType

    Hp, Wp = H + 2, W + 2
    S_pad = Hp * Wp
    S = H * W
    DOFF = 32  # left zero pad inside each channel-chunk row buffer
    S_chunk = ((S_pad + DOFF + 32 + 15) // 16) * 16  # right pad >= 32
    NQ = 4  # prep groups (and img-0 load quarters) per image
    GR = H // NQ  # rows per prep group
    assert H % NQ == 0
    PO = (H // 2) * (W // 2)
    WH = W // 2

    # conv2 row groups (pool-pair aligned)
    max_rows = (psum_stretch // Wp) // 2 * 2
    row_groups = []
    r = 0
    while r < H:
        g = min(max_rows, H - r)
        row_groups.append((r, g))
        r += g
    st2 = [((1 + r0) * Wp, rg * Wp, r0, rg) for r0, rg in row_groups]
    # last image: split the final row group so the pool/store chain after
    # the very last matmul is as short as possible
    tail_rows = list(row_groups[:-1])
    lr0, lrg = row_groups[-1]
    if tail_split and lrg > tail_split:
        tail_rows += [(lr0, lrg - tail_split), (lr0 + lrg - tail_split, tail_split)]
    else:
        tail_rows.append((lr0, lrg))
    st2_tail = [((1 + r0) * Wp, rg * Wp, r0, rg) for r0, rg in tail_rows]
    # image 0 is prepped in 7-row pieces (NP1 of them) with conv1 stretches
    # aligned so stretch s only reads input rows loaded by pieces <= s. The
    # +Wp+1 shifted window spills one byte into the row after r0+rg, so
    # reserve one extra row per boundary.
    # pieces: two 7-row halves of the first quarter, then whole quarters.
    # stretch s may read up to one row past its end, so each stretch stops
    # two rows short of its piece's cumulative coverage.
    GRH = GR // 2
    p0 = [(k * GRH, GRH) for k in range(4)] + [
        (GR * k, GR) for k in range(2, NQ)
    ]
    rg1 = []
    r = 0
    cum = 0
    for i, (plo, pn) in enumerate(p0):
        cum += pn
        hi = H if i == len(p0) - 1 else cum - 2
        rg1.append((r, hi - r))
        r = hi
    NP1 = len(p0)
    st1_first = [((1 + r0) * Wp, rg * Wp, r0, rg) for r0, rg in rg1]
    st1_rest = st2
    PS_COLS = psum_stretch

    nc = bacc.Bacc("TRN2", target_bir_lowering=False, debug=False)

    x_h = nc.dram_tensor("x", [B, C, S], F32, kind="ExternalInput")
    cb_h = nc.dram_tensor("cb", [KP, CONST_B], U8, kind="ExternalInput")
    y_h = nc.dram_tensor("y", [B, NCHUNK, KP, PO], F32, kind="ExternalOutput")

    def dram_ap(handle, offset, dims):
        return bass.AP(
            tensor=handle.ap().tensor, offset=offset, ap=[list(d) for d in dims]
        )

    with tile.TileContext(nc) as tc:
        from contextlib import ExitStack

        with ExitStack() as ctx:
            consts = ctx.enter_context(tc.tile_pool(name="consts", bufs=1))
            xnat_p = ctx.enter_context(tc.tile_pool(name="xnat", bufs=1))
            xsT_p = ctx.enter_context(tc.tile_pool(name="xsT", bufs=2))
            hsT_p = ctx.enter_context(tc.tile_pool(name="hsT", bufs=2))
            pr_p = ctx.enter_context(tc.tile_pool(name="prp", bufs=2))
            po_p = ctx.enter_context(tc.tile_pool(name="pop", bufs=2))
            convp = ctx.enter_context(
                tc.tile_pool(name="convp", bufs=conv_bufs, space="PSUM")
            )

            # --- packed constants (three DMAs: w1-j0+nt1, w1-j1, rest)
            cb = consts.tile([KP, CONST_B], U8)
            w1j = [
                cb[:, W1J0_OFF : W1J0_OFF + 2304].bitcast(FP8).rearrange(
                    "p (t k m) -> p t k m", t=9, k=2
                ),
                cb[:, W1J1_OFF : W1J1_OFF + 2304].bitcast(FP8).rearrange(
                    "p (t k m) -> p t k m", t=9, k=2
                ),
            ]
            w2sb = cb[:, W2_OFF : W2_OFF + 4608].bitcast(FP8).rearrange(
                "p (t j k m) -> p t j k m", t=9, j=NCHUNK, k=2
            )
            w1v = lambda j, t: w1j[j][:, t]
            w2v = lambda j, t: w2sb[:, t, j]
            nt1sb = cb[:, NT1_OFF : NT1_OFF + 8].bitcast(F32)
            s2sb = cb[:, S2_OFF : S2_OFF + 8].bitcast(F32)
            b2sb = cb[:, B2_OFF : B2_OFF + 8].bitcast(F32)

            # --- preload the ACT piecewise-poly table (Sign) with a tiny
            # dependency-free activation so the 1.3us table load is off the
            # critical prep chain
            dummy = consts.tile([1, 4], F32)
            nc.vector.memset(dummy, 0.0)
            nc.scalar.activation(dummy, dummy, Act.Sign, bias=0.0, scale=1.0)

            from concourse import masks

            id8sb = consts.tile([KP, KP], BF16)
            masks.make_identity(nc, id8sb)

            # --- PE p-state warmup: dependency-free junk transposes keep the
            # tensor engine busy from t~0 so the ramp is spent before real
            # conv matmuls arrive. The junk lives in a convp rotation buffer
            # (conv matmuls start=True overwrite it later).
            if warm_mm:
                warm = convp.tile([KP, KP], BF16, tag="cv", name="warm")
                for _ in range(warm_mm):
                    nc.tensor.transpose(warm, id8sb, id8sb)

            # --- loads, issued in priority order (DMA transfers serialize)
            xn = {}
            for img in range(B):
                xn[img] = xnat_p.tile(
                    [KP, NCHUNK, S], F32, tag=f"xn{img}", name=f"xn{img}"
                )

            def load_x_span(img, s0, s1):
                nc.sync.dma_start(
                    out=xn[img][:, :, s0:s1],
                    in_=dram_ap(
                        x_h,
                        img * C * S + s0,
                        [[S, KP], [KP * S, NCHUNK], [1, s1 - s0]],
                    ),
                )

            Q = GR * W   # spatial elems per steady-state prep quarter
            load_x_span(0, 0, GRH * W)
            nc.sync.dma_start(out=cb[:, :CBA_B], in_=cb_h.ap()[:, :CBA_B])
            load_x_span(0, GRH * W, GR * W)
            nc.sync.dma_start(out=cb[:, CBA_B:CBB_B], in_=cb_h.ap()[:, CBA_B:CBB_B])
            for plo, pn in p0[2:]:
                load_x_span(0, plo * W, (plo + pn) * W)
            nc.sync.dma_start(out=cb[:, CBB_B:], in_=cb_h.ap()[:, CBB_B:])
            for img in range(1, B):
                load_x_span(img, 0, S // 2)
                load_x_span(img, S // 2, S)

            # --- helpers
            def border_memsets(buf):
                # rows 0 and H+1, left/right pads, and border cols {0, W+1} of
                # rows 1..H; on GPSIMD so the vector engines stay free.
                nc.gpsimd.memset(buf[:, :, 0 : DOFF + Wp], 0.0)
                nc.gpsimd.memset(buf[:, :, DOFF + (H + 1) * Wp : S_chunk], 0.0)
                rows = buf[:, :, DOFF + Wp : DOFF + (H + 1) * Wp].rearrange(
                    "p j (r w) -> p j r w", w=Wp
                )
                nc.gpsimd.memset(rows[:, :, :, 0 :: (W + 1)], 0.0)

            xsT_tiles = {}

            def prep_span(img, lo, hi):
                # fused sign+scatter of rows [lo, hi): fp32 -> fp8 +-0.5
                # written straight into the padded conv layout; j0 on DVE,
                # j1 on GPSIMD so the two planes run in parallel
                if lo == 0:
                    xsT_tiles[img] = xsT_p.tile(
                        [KP, NCHUNK, S_chunk], FP8, tag="xsT", name=f"xsT{img}"
                    )
                    border_memsets(xsT_tiles[img])
                xsT = xsT_tiles[img]
                a0 = DOFF + (1 + lo) * Wp
                for j in range(NCHUNK):
                    src = xn[img][:, j, lo * W : hi * W].rearrange(
                        "p (r w) -> p r w", w=W
                    )
                    dst = xsT[:, j, a0 : a0 + (hi - lo) * Wp].rearrange(
                        "p (r w) -> p r w", w=Wp
                    )[:, :, 1 : 1 + W]
                    eng = nc.vector if j == 0 else nc.gpsimd
                    eng.tensor_scalar(
                        dst, src, 0.0, 0.5, Alu.is_ge, Alu.subtract
                    )

            def prep_group(img, g):
                prep_span(img, g * GR, (g + 1) * GR)

            def conv_stretch(inbuf, wv, st, si, j, psum_cb, nm):
                cs, cn, r0, rg = st
                ps = convp.tile([KP, PS_COLS], F32, tag="cv", name=f"cv{nm}{si}{j}")
                if exact_rows:
                    # per-row 56-col matmuls (skip the 2 pad cols per row),
                    # row-outer/tap-inner so each row's PSUM accumulation
                    # group opens and closes before the next row touches the
                    # same 2KB bank. Rows sit at a 64-col pitch so no matmul
                    # output straddles a bank.
                    for r in range(rg):
                        for t in range(9):
                            dy, dx = t // 3, t % 3
                            a = DOFF + (r0 + r + dy) * Wp + dx
                            nc.tensor.matmul(
                                ps[:, r * 64 : r * 64 + W],
                                wv(j, t),
                                inbuf[:, :, a : a + W],
                                start=(t == 0),
                                stop=(t == 8),
                                perf_mode=DR,
                            )
                else:
                    for t in range(9):
                        dy, dx = t // 3, t % 3
                        lhsT = wv(j, t)
                        off = (dy - 1) * Wp + (dx - 1)
                        for c0 in range(0, cn, 512):
                            n = min(512, cn - c0)
                            a = DOFF + cs + off + c0
                            nc.tensor.matmul(
                                ps[:, c0 : c0 + n],
                                lhsT,
                                inbuf[:, :, a : a + n],
                                start=(t == 0),
                                stop=(t == 8),
                                perf_mode=DR,
                            )
                psum_cb(si, j, ps, st)

            hsT_tiles = {}

            def conv1_stretch(img, si):
                sts1 = st1_first if img == 0 else st1_rest
                st = sts1[si]
                if si == 0:
                    hsT_tiles[img] = hsT_p.tile(
                        [KP, NCHUNK, S_chunk], FP8, tag="hsT", name=f"hsT{img}"
                    )
                    border_memsets(hsT_tiles[img])
                hsT = hsT_tiles[img]

                def bnsign(si_, j, ps, st_):
                    cs, cn, r0_, rg_ = st_
                    dstv = hsT[:, j, DOFF + cs : DOFF + cs + cn].rearrange(
                        "p (r w) -> p r w", w=Wp
                    )[:, :, 1 : 1 + W]
                    if exact_rows:
                        srcv = ps[:, : rg_ * 64].rearrange("p (r w) -> p r w", w=64)[
                            :, :, :W
                        ]
                    else:
                        srcv = ps[:, :cn].rearrange("p (r w) -> p r w", w=Wp)[
                            :, :, 1 : 1 + W
                        ]
                    nc.scalar.activation(
                        dstv, srcv, Act.Sign, bias=nt1sb[:, j : j + 1], scale=1.0
                    )

                for j in range(NCHUNK):
                    conv_stretch(xsT_tiles[img], w1v, st, si, j, bnsign, f"a{img}")
                if si == len(sts1) - 1:
                    xsT_tiles.pop(img)

            pr_tiles = {}
            pooled_tiles = {}

            def conv2_stretch(img, si):
                sts = st2_tail if img == B - 1 else st2
                st = sts[si]
                if si == 0:
                    pr_tiles[img] = [
                        pr_p.tile([KP, H // 2, W], F32, tag="pr", name=f"pr{img}{j}")
                        for j in range(NCHUNK)
                    ]
                    pooled_tiles[img] = [
                        po_p.tile([KP, PO], F32, tag="pooled", name=f"pl{img}{j}")
                        for j in range(NCHUNK)
                    ]

                def pool_cb(si_, j, ps, st_):
                    cs, cn, r0, rg = st_
                    if exact_rows:
                        rows = ps[:, : rg * 64].rearrange("p (q t) -> p q t", t=128)
                        in0 = rows[:, :, 0:W]
                        in1 = rows[:, :, 64 : 64 + W]
                    else:
                        rows = ps[:, : rg * Wp].rearrange("p (q t) -> p q t", t=2 * Wp)
                        in0 = rows[:, :, 1 : 1 + W]
                        in1 = rows[:, :, Wp + 1 : Wp + 1 + W]
                    q0, q1 = r0 // 2, (r0 + rg) // 2
                    q = rg // 2
                    # TensorTensor may read only one input from PSUM: stage
                    # the even rows into SBUF, then max against the PSUM odd
                    # rows. Steady state splits the copy to ACT for engine
                    # parallelism; the last image's short tail chains run
                    # entirely on DVE to avoid cross-engine sem hops.
                    prA = pr_p.tile(
                        [KP, max_rows // 2, W], F32, tag="prA", bufs=4,
                        name=f"prA{img}{si_}{j}",
                    )
                    nc.scalar.copy(prA[:, :q, :], in0)
                    nc.vector.tensor_max(
                        pr_tiles[img][j][:, q0:q1, :], prA[:, :q, :], in1
                    )
                    prs = pr_tiles[img][j][:, q0:q1, :].rearrange("p q w -> p (q w)")
                    pv = pooled_tiles[img][j].rearrange("p (q w) -> p q w", w=WH)[
                        :, q0:q1, :
                    ]
                    nc.vector.tensor_max(pv, prs[:, 0::2], prs[:, 1::2])
                    nc.vector.tensor_scalar(
                        pv, pv, s2sb[:, j : j + 1], b2sb[:, j : j + 1],
                        Alu.mult, Alu.add,
                    )
                    # stores: whole channel-chunk per image, but per-stretch
                    # for the last image so the tail ships immediately
                    if img == B - 1:
                        nc.sync.dma_start(
                            out=dram_ap(
                                y_h,
                                (img * NCHUNK + j) * KP * PO + q0 * WH,
                                [[PO, KP], [1, (q1 - q0) * WH]],
                            ),
                            in_=pooled_tiles[img][j][:, q0 * WH : q1 * WH],
                        )
                    elif si_ == len(sts) - 1:
                        nc.sync.dma_start(
                            out=dram_ap(
                                y_h,
                                (img * NCHUNK + j) * KP * PO,
                                [[PO, KP], [1, PO]],
                            ),
                            in_=pooled_tiles[img][j],
                        )

                for j in range(NCHUNK):
                    conv_stretch(hsT_tiles[img], w2v, st, si, j, pool_cb, f"b{img}")
                if si == len(sts) - 1:
                    hsT_tiles.pop(img)

            # --- emission ---
            # image 0: prep each 7-row piece right before the conv1 stretch
            # that needs it. The conv rhs spans both channel planes as one
            # interval hull, so any copy emitted before a stretch becomes a
            # dependency of it — never emit a copy ahead of an earlier
            # stretch.
            for k in range(NP1):
                plo, pn = p0[k]
                prep_span(0, plo, plo + pn)
                conv1_stretch(0, k)
            for img in range(B):
                if img > 0:
                    for si in range(len(st2)):
                        conv1_stretch(img, si)
                        if img + 1 < B:
                            prep_group(img + 1, si)
                for si in range(len(st2_tail if img == B - 1 else st2)):
                    if img == 0 and B > 1 and si < NQ:
                        prep_group(1, si)
                    conv2_stretch(img, si)

    nc.compile()
    return nc


# ---------------------------------------------------------------------------
# host-side constant prep
# ---------------------------------------------------------------------------


def _prep_consts(w1, beta1, mean1, var1, w2, beta2, mean2, var2):
    import jax
    import jax.numpy as jnp
    from jax import lax
    from concourse import mybir

    fp8np = mybir.dt.np(mybir.dt.float8e4)

    def prep_w(w, j_major=False):
        ws = np.where(np.asarray(w) >= 0, np.float32(1.0), np.float32(-1.0))
        # [3,3,ci,co] -> [p, (j,) tap, ktile, m]; ci = ktile*128+p, co = j*128+m
        wr = ws.reshape(9, 2, KP, NCHUNK, KP)
        wr = wr.transpose((2, 3, 0, 1, 4) if j_major else (2, 0, 3, 1, 4))
        return np.ascontiguousarray(wr).astype(fp8np)

    w1p, w2p = prep_w(w1, j_major=True), prep_w(w2)

    cpu = jax.devices("cpu")[0]
    MAXH = 9 * C
    with jax.default_device(cpu):
        hs = jnp.arange(-MAXH, MAXH + 1, dtype=jnp.float32)
        bn1 = (hs[:, None] - jnp.asarray(mean1)[None, :]) * lax.rsqrt(
            jnp.asarray(var1) + 1e-3
        )[None, :] + jnp.asarray(beta1)[None, :]
        nonneg = np.asarray(bn1 >= 0)
        r2 = np.asarray(lax.rsqrt(jnp.asarray(var2) + 1e-3))

    assert (np.diff(nonneg.astype(np.int8), axis=0) >= 0).all(), "bn1 not monotone"
    kc = np.where(nonneg.any(0), nonneg.argmax(0), 2 * MAXH + 1) - MAXH
    # device psum holds h/2 (x=+-0.5, w=+-1): sign flips at (kc-0.5)/2
    nt1 = (-(kc.astype(np.float64) - 0.5) / 2.0).astype(np.float32)

    s2 = r2.astype(np.float32)
    b2 = (
        np.asarray(beta2, np.float64)
        - np.asarray(mean2, np.float64) * s2.astype(np.float64)
    ).astype(np.float32)

    def to_pj(a):  # [256] -> [128, 2] with c = j*128+p
        return np.ascontiguousarray(a.reshape(NCHUNK, KP).T).astype(np.float32)

    # pack everything into one [128, CONST_B] uint8 image
    cbuf = np.zeros((KP, CONST_B), dtype=np.uint8)

    def put(off, arr):
        by = np.ascontiguousarray(arr).reshape(KP, -1).view(np.uint8)
        cbuf[:, off : off + by.shape[1]] = by

    put(W1J0_OFF, w1p[:, 0])
    put(W1J1_OFF, w1p[:, 1])
    put(W2_OFF, w2p)
    put(NT1_OFF, to_pj(nt1))
    put(S2_OFF, to_pj(s2))
    put(B2_OFF, to_pj(b2))
    return {"cb": cbuf}


# ---------------------------------------------------------------------------
# entry point
# ---------------------------------------------------------------------------

_cached = {}


def _run(inputs, trace=False):
    from concourse import bass_utils

    x = np.asarray(inputs["x"], dtype=np.float32)
    Bt, H, W, _ = x.shape  # 32, 56, 56, 256
    Bc = Bt // N_CORES

    consts = _prep_consts(
        inputs["w1"], inputs["beta1"], inputs["mean1"], inputs["var1"],
        inputs["w2"], inputs["beta2"], inputs["mean2"], inputs["var2"],
    )

    key = (Bc, H, W)
    if key not in _cached:
        _cached[key] = build_program(Bc, H, W)
    nc = _cached[key]

    # channel-major marshaling: [Bt, H*W, C] -> per-core [Bc, C, H*W]
    xcm = np.ascontiguousarray(
        x.reshape(Bt, H * W, C).transpose(0, 2, 1)
    )

    in_maps = []
    for c in range(N_CORES):
        m = dict(consts)
        m["x"] = xcm[c * Bc : (c + 1) * Bc]
        in_maps.append(m)

    res = bass_utils.run_bass_kernel_spmd(
        nc, in_maps, core_ids=list(range(N_CORES)), trace=trace
    )
    PO = (H // 2) * (W // 2)
    # y comes back channel-major [Bc, 2, 128, PO]; restore NHWC
    y = np.concatenate(
        [r["y"].reshape(Bc, C, PO).transpose(0, 2, 1) for r in res.results], axis=0
    )
    y = np.ascontiguousarray(y.reshape(Bt, H // 2, W // 2, C)).astype(np.float32)
    return y, res


def kernel(**inputs):
    y, _ = _run(inputs, trace=False)
    return y


# revision 55
# speedup vs baseline: 1.3018x; 1.0020x over previous
"""Trainium2 Bass kernel for a BinaryNet conv block.

Pipeline (per core, data-parallel over batch):
  sign(x) -> conv3x3(sign(w1)) -> BN1 -> sign -> conv3x3(sign(w2))
          -> maxpool2x2 -> BN2

Implementation notes:
  - Activations are +-0.5, weights +-1.0 in fp8e4 (exactly representable);
    convs run as 9 shifted-window matmuls with DoubleRow perf mode (K=256
    contraction per instruction), accumulating exactly into fp32 PSUM.
  - BN1+sign is fused into one ScalarE Sign activation against a
    host-precomputed per-channel threshold. Conv outputs are exact
    integers, so an integer cutoff k_c reproduces the reference's fp32
    sign decisions bit-exactly.
  - The host marshals x to channel-major [C, H*W] per image and reads y
    back channel-major [2, 128, PO]; the device never transposes. The PE
    therefore runs conv matmuls only, fed by DVE sign + ACT/DVE scatter
    copies into the zero-bordered padded layout.
  - Spatial layout is channel-major [ci, y*(W+2)+x] with a zero border so
    the 9 taps are just constant AP offsets.
  - The bass2jax/pseudo-DMA path allows only ONE sync wait per DMA; every
    DMA destination is a fresh tile (or a disjoint slice of one), so no
    DMA ever needs more than one semaphore wait. All loads are issued
    up-front in priority order (DMA transfers serialize), stores as
    produced.
  - A short burst of junk transposes warms the PE p-state ramp so the
    first real conv matmuls run at full clock.
"""

import os
import numpy as np

os.environ.setdefault("MYCRO_LOCAL_CACHE", "1")

N_CORES = 8
C = 256
NCHUNK = 2  # channel chunks of 128
KP = 128

# packed consts layout (bytes per partition); w1 split by output-channel
# chunk so the first conv can start as soon as the j0 half lands
W1J0_OFF = 0        # fp8 [9,2,128] -> 2304 B
NT1_OFF = 2304      # f32 [2] -> 8 B
CBA_B = 2312        # first consts DMA covers [0, CBA_B)
W1J1_OFF = 2312     # fp8 -> 2304 B
CBB_B = 4616        # second consts DMA covers [CBA_B, CBB_B)
W2_OFF = 4616       # fp8 [9,2,2,128] -> 4608 B
S2_OFF = 9224       # f32 [2]
B2_OFF = 9232       # f32 [2]
CONST_B = 9240


def build_program(B, H, W, psum_stretch=1024, conv_bufs=4, warm_mm=46, tail_split=0, exact_rows=True):
    """Build the per-core Bass program. B images of HxWxC per core."""
    import concourse.bass as bass
    import concourse.bacc as bacc
    import concourse.tile as tile
    from concourse import mybir

    F32 = mybir.dt.float32
    FP8 = mybir.dt.float8e4
    BF16 = mybir.dt.bfloat16
    U8 = mybir.dt.uint8
    DR = mybir.MatmulPerfMode.DoubleRow
    Alu = mybir.AluOpType
    Act = mybir.ActivationFunctionType

    Hp, Wp = H + 2, W + 2
    S_pad = Hp * Wp
    S = H * W
    DOFF = 32  # left zero pad inside each channel-chunk row buffer
    S_chunk = ((S_pad + DOFF + 32 + 15) // 16) * 16  # right pad >= 32
    NQ = 4  # prep groups (and img-0 load quarters) per image
    GR = H // NQ  # rows per prep group
    assert H % NQ == 0
    PO = (H // 2) * (W // 2)
    WH = W // 2

    # conv2 row groups (pool-pair aligned)
    max_rows = (psum_stretch // Wp) // 2 * 2
    row_groups = []
    r = 0
    while r < H:
        g = min(max_rows, H - r)
        row_groups.append((r, g))
        r += g
    st2 = [((1 + r0) * Wp, rg * Wp, r0, rg) for r0, rg in row_groups]
    # last image: split the final row group so the pool/store chain after
    # the very last matmul is as short as possible
    tail_rows = list(row_groups[:-1])
    lr0, lrg = row_groups[-1]
    if tail_split and lrg > tail_split:
        tail_rows += [(lr0, lrg - tail_split), (lr0 + lrg - tail_split, tail_split)]
    else:
        tail_rows.append((lr0, lrg))
    st2_tail = [((1 + r0) * Wp, rg * Wp, r0, rg) for r0, rg in tail_rows]
    # image 0 is prepped in 7-row pieces (NP1 of them) with conv1 stretches
    # aligned so stretch s only reads input rows loaded by pieces <= s. The
    # +Wp+1 shifted window spills one byte into the row after r0+rg, so
    # reserve one extra row per boundary.
    # pieces: two 7-row halves of the first quarter, then whole quarters.
    # stretch s may read up to one row past its end, so each stretch stops
    # two rows short of its piece's cumulative coverage.
    GRH = GR // 2
    p0 = [(k * GRH, GRH) for k in range(4)] + [
        (GR * k, GR) for k in range(2, NQ)
    ]
    rg1 = []
    r = 0
    cum = 0
    for i, (plo, pn) in enumerate(p0):
        cum += pn
        hi = H if i == len(p0) - 1 else cum - 2
        rg1.append((r, hi - r))
        r = hi
    NP1 = len(p0)
    st1_first = [((1 + r0) * Wp, rg * Wp, r0, rg) for r0, rg in rg1]
    st1_rest = st2
    PS_COLS = psum_stretch

    nc = bacc.Bacc("TRN2", target_bir_lowering=False, debug=False)

    x_h = nc.dram_tensor("x", [B, C, S], F32, kind="ExternalInput")
    cb_h = nc.dram_tensor("cb", [KP, CONST_B], U8, kind="ExternalInput")
    y_h = nc.dram_tensor("y", [B, NCHUNK, KP, PO], F32, kind="ExternalOutput")

    def dram_ap(handle, offset, dims):
        return bass.AP(
            tensor=handle.ap().tensor, offset=offset, ap=[list(d) for d in dims]
        )

    with tile.TileContext(nc) as tc:
        from contextlib import ExitStack

        with ExitStack() as ctx:
            consts = ctx.enter_context(tc.tile_pool(name="consts", bufs=1))
            xnat_p = ctx.enter_context(tc.tile_pool(name="xnat", bufs=1))
            xsT_p = ctx.enter_context(tc.tile_pool(name="xsT", bufs=2))
            hsT_p = ctx.enter_context(tc.tile_pool(name="hsT", bufs=2))
            pr_p = ctx.enter_context(tc.tile_pool(name="prp", bufs=2))
            po_p = ctx.enter_context(tc.tile_pool(name="pop", bufs=2))
            convp = ctx.enter_context(
                tc.tile_pool(name="convp", bufs=conv_bufs, space="PSUM")
            )

            # --- packed constants (three DMAs: w1-j0+nt1, w1-j1, rest)
            cb = consts.tile([KP, CONST_B], U8)
            w1j = [
                cb[:, W1J0_OFF : W1J0_OFF + 2304].bitcast(FP8).rearrange(
                    "p (t k m) -> p t k m", t=9, k=2
                ),
                cb[:, W1J1_OFF : W1J1_OFF + 2304].bitcast(FP8).rearrange(
                    "p (t k m) -> p t k m", t=9, k=2
                ),
            ]
            w2sb = cb[:, W2_OFF : W2_OFF + 4608].bitcast(FP8).rearrange(
                "p (t j k m) -> p t j k m", t=9, j=NCHUNK, k=2
            )
            w1v = lambda j, t: w1j[j][:, t]
            w2v = lambda j, t: w2sb[:, t, j]
            nt1sb = cb[:, NT1_OFF : NT1_OFF + 8].bitcast(F32)
            s2sb = cb[:, S2_OFF : S2_OFF + 8].bitcast(F32)
            b2sb = cb[:, B2_OFF : B2_OFF + 8].bitcast(F32)

            # --- preload the ACT piecewise-poly table (Sign) with a tiny
            # dependency-free activation so the 1.3us table load is off the
            # critical prep chain
            dummy = consts.tile([1, 4], F32)
            nc.vector.memset(dummy, 0.0)
            nc.scalar.activation(dummy, dummy, Act.Sign, bias=0.0, scale=1.0)

            from concourse import masks

            id8sb = consts.tile([KP, KP], BF16)
            masks.make_identity(nc, id8sb)

            # --- PE p-state warmup: dependency-free junk transposes keep the
            # tensor engine busy from t~0 so the ramp is spent before real
            # conv matmuls arrive. The junk lives in a convp rotation buffer
            # (conv matmuls start=True overwrite it later).
            if warm_mm:
                warm = convp.tile([KP, KP], BF16, tag="cv", name="warm")
                for _ in range(warm_mm):
                    nc.tensor.transpose(warm, id8sb, id8sb)

            # --- loads, issued in priority order (DMA transfers serialize)
            xn = {}
            for img in range(B):
                xn[img] = xnat_p.tile(
                    [KP, NCHUNK, S], F32, tag=f"xn{img}", name=f"xn{img}"
                )

            def load_x_span(img, s0, s1):
                nc.sync.dma_start(
                    out=xn[img][:, :, s0:s1],
                    in_=dram_ap(
                        x_h,
                        img * C * S + s0,
                        [[S, KP], [KP * S, NCHUNK], [1, s1 - s0]],
                    ),
                )

            Q = GR * W   # spatial elems per steady-state prep quarter
            load_x_span(0, 0, GRH * W)
            nc.sync.dma_start(out=cb[:, :CBA_B], in_=cb_h.ap()[:, :CBA_B])
            load_x_span(0, GRH * W, GR * W)
            nc.sync.dma_start(out=cb[:, CBA_B:CBB_B], in_=cb_h.ap()[:, CBA_B:CBB_B])
            for plo, pn in p0[2:]:
                load_x_span(0, plo * W, (plo + pn) * W)
            nc.sync.dma_start(out=cb[:, CBB_B:], in_=cb_h.ap()[:, CBB_B:])
            for img in range(1, B):
                load_x_span(img, 0, S // 2)
                load_x_span(img, S // 2, S)

            # --- helpers
            def border_memsets(buf):
                # rows 0 and H+1, left/right pads, and border cols {0, W+1} of
                # rows 1..H; on GPSIMD so the vector engines stay free.
                nc.gpsimd.memset(buf[:, :, 0 : DOFF + Wp], 0.0)
                nc.gpsimd.memset(buf[:, :, DOFF + (H + 1) * Wp : S_chunk], 0.0)
                rows = buf[:, :, DOFF + Wp : DOFF + (H + 1) * Wp].rearrange(
                    "p j (r w) -> p j r w", w=Wp
                )
                nc.gpsimd.memset(rows[:, :, :, 0 :: (W + 1)], 0.0)

            xsT_tiles = {}

            def prep_span(img, lo, hi, all_dve=False):
                # fused sign+scatter of rows [lo, hi): fp32 -> fp8 +-0.5
                # written straight into the padded conv layout; j0 on DVE,
                # j1 on GPSIMD so the two planes run in parallel (image 0:
                # both on DVE, whose op is 2.4x faster than GPSIMD's)
                if lo == 0:
                    xsT_tiles[img] = xsT_p.tile(
                        [KP, NCHUNK, S_chunk], FP8, tag="xsT", name=f"xsT{img}"
                    )
                    border_memsets(xsT_tiles[img])
                xsT = xsT_tiles[img]
                a0 = DOFF + (1 + lo) * Wp
                for j in range(NCHUNK):
                    src = xn[img][:, j, lo * W : hi * W].rearrange(
                        "p (r w) -> p r w", w=W
                    )
                    dst = xsT[:, j, a0 : a0 + (hi - lo) * Wp].rearrange(
                        "p (r w) -> p r w", w=Wp
                    )[:, :, 1 : 1 + W]
                    eng = nc.vector if (j == 0 or all_dve) else nc.gpsimd
                    eng.tensor_scalar(
                        dst, src, 0.0, 0.5, Alu.is_ge, Alu.subtract
                    )

            def prep_group(img, g):
                prep_span(img, g * GR, (g + 1) * GR)

            def conv_stretch(inbuf, wv, st, si, j, psum_cb, nm):
                cs, cn, r0, rg = st
                ps = convp.tile([KP, PS_COLS], F32, tag="cv", name=f"cv{nm}{si}{j}")
                if exact_rows:
                    # per-row 56-col matmuls (skip the 2 pad cols per row),
                    # row-outer/tap-inner so each row's PSUM accumulation
                    # group opens and closes before the next row touches the
                    # same 2KB bank. Rows sit at a 64-col pitch so no matmul
                    # output straddles a bank.
                    for r in range(rg):
                        for t in range(9):
                            dy, dx = t // 3, t % 3
                            a = DOFF + (r0 + r + dy) * Wp + dx
                            nc.tensor.matmul(
                                ps[:, r * 64 : r * 64 + W],
                                wv(j, t),
                                inbuf[:, :, a : a + W],
                                start=(t == 0),
                                stop=(t == 8),
                                perf_mode=DR,
                            )
                else:
                    for t in range(9):
                        dy, dx = t // 3, t % 3
                        lhsT = wv(j, t)
                        off = (dy - 1) * Wp + (dx - 1)
                        for c0 in range(0, cn, 512):
                            n = min(512, cn - c0)
                            a = DOFF + cs + off + c0
                            nc.tensor.matmul(
                                ps[:, c0 : c0 + n],
                                lhsT,
                                inbuf[:, :, a : a + n],
                                start=(t == 0),
                                stop=(t == 8),
                                perf_mode=DR,
                            )
                psum_cb(si, j, ps, st)

            hsT_tiles = {}

            def conv1_stretch(img, si):
                sts1 = st1_first if img == 0 else st1_rest
                st = sts1[si]
                if si == 0:
                    hsT_tiles[img] = hsT_p.tile(
                        [KP, NCHUNK, S_chunk], FP8, tag="hsT", name=f"hsT{img}"
                    )
                    border_memsets(hsT_tiles[img])
                hsT = hsT_tiles[img]

                def bnsign(si_, j, ps, st_):
                    cs, cn, r0_, rg_ = st_
                    dstv = hsT[:, j, DOFF + cs : DOFF + cs + cn].rearrange(
                        "p (r w) -> p r w", w=Wp
                    )[:, :, 1 : 1 + W]
                    if exact_rows:
                        srcv = ps[:, : rg_ * 64].rearrange("p (r w) -> p r w", w=64)[
                            :, :, :W
                        ]
                    else:
                        srcv = ps[:, :cn].rearrange("p (r w) -> p r w", w=Wp)[
                            :, :, 1 : 1 + W
                        ]
                    nc.scalar.activation(
                        dstv, srcv, Act.Sign, bias=nt1sb[:, j : j + 1], scale=1.0
                    )

                for j in range(NCHUNK):
                    conv_stretch(xsT_tiles[img], w1v, st, si, j, bnsign, f"a{img}")
                if si == len(sts1) - 1:
                    xsT_tiles.pop(img)

            pr_tiles = {}
            pooled_tiles = {}

            def conv2_stretch(img, si):
                sts = st2_tail if img == B - 1 else st2
                st = sts[si]
                if si == 0:
                    pr_tiles[img] = [
                        pr_p.tile([KP, H // 2, W], F32, tag="pr", name=f"pr{img}{j}")
                        for j in range(NCHUNK)
                    ]
                    pooled_tiles[img] = [
                        po_p.tile([KP, PO], F32, tag="pooled", name=f"pl{img}{j}")
                        for j in range(NCHUNK)
                    ]

                def pool_cb(si_, j, ps, st_):
                    cs, cn, r0, rg = st_
                    if exact_rows:
                        rows = ps[:, : rg * 64].rearrange("p (q t) -> p q t", t=128)
                        in0 = rows[:, :, 0:W]
                        in1 = rows[:, :, 64 : 64 + W]
                    else:
                        rows = ps[:, : rg * Wp].rearrange("p (q t) -> p q t", t=2 * Wp)
                        in0 = rows[:, :, 1 : 1 + W]
                        in1 = rows[:, :, Wp + 1 : Wp + 1 + W]
                    q0, q1 = r0 // 2, (r0 + rg) // 2
                    q = rg // 2
                    # TensorTensor may read only one input from PSUM: stage
                    # the even rows into SBUF, then max against the PSUM odd
                    # rows. Steady state splits the copy to ACT for engine
                    # parallelism; the last image's short tail chains run
                    # entirely on DVE to avoid cross-engine sem hops.
                    prA = pr_p.tile(
                        [KP, max_rows // 2, W], F32, tag="prA", bufs=4,
                        name=f"prA{img}{si_}{j}",
                    )
                    nc.scalar.copy(prA[:, :q, :], in0)
                    nc.vector.tensor_max(
                        pr_tiles[img][j][:, q0:q1, :], prA[:, :q, :], in1
                    )
                    prs = pr_tiles[img][j][:, q0:q1, :].rearrange("p q w -> p (q w)")
                    pv = pooled_tiles[img][j].rearrange("p (q w) -> p q w", w=WH)[
                        :, q0:q1, :
                    ]
                    nc.vector.tensor_max(pv, prs[:, 0::2], prs[:, 1::2])
                    nc.vector.tensor_scalar(
                        pv, pv, s2sb[:, j : j + 1], b2sb[:, j : j + 1],
                        Alu.mult, Alu.add,
                    )
                    # stores: whole channel-chunk per image, but per-stretch
                    # for the last image so the tail ships immediately
                    if img == B - 1:
                        nc.sync.dma_start(
                            out=dram_ap(
                                y_h,
                                (img * NCHUNK + j) * KP * PO + q0 * WH,
                                [[PO, KP], [1, (q1 - q0) * WH]],
                            ),
                            in_=pooled_tiles[img][j][:, q0 * WH : q1 * WH],
                        )
                    elif si_ == len(sts) - 1:
                        nc.sync.dma_start(
                            out=dram_ap(
                                y_h,
                                (img * NCHUNK + j) * KP * PO,
                                [[PO, KP], [1, PO]],
                            ),
                            in_=pooled_tiles[img][j],
                        )

                for j in range(NCHUNK):
                    conv_stretch(hsT_tiles[img], w2v, st, si, j, pool_cb, f"b{img}")
                if si == len(sts) - 1:
                    hsT_tiles.pop(img)

            # --- emission ---
            # image 0: prep each 7-row piece right before the conv1 stretch
            # that needs it. The conv rhs spans both channel planes as one
            # interval hull, so any copy emitted before a stretch becomes a
            # dependency of it — never emit a copy ahead of an earlier
            # stretch.
            for k in range(NP1):
                plo, pn = p0[k]
                prep_span(0, plo, plo + pn, all_dve=True)
                conv1_stretch(0, k)
            for img in range(B):
                if img > 0:
                    for si in range(len(st2)):
                        conv1_stretch(img, si)
                        if img + 1 < B:
                            prep_group(img + 1, si)
                for si in range(len(st2_tail if img == B - 1 else st2)):
                    if img == 0 and B > 1 and si < NQ:
                        prep_group(1, si)
                    conv2_stretch(img, si)

    nc.compile()
    return nc


# ---------------------------------------------------------------------------
# host-side constant prep
# ---------------------------------------------------------------------------


def _prep_consts(w1, beta1, mean1, var1, w2, beta2, mean2, var2):
    import jax
    import jax.numpy as jnp
    from jax import lax
    from concourse import mybir

    fp8np = mybir.dt.np(mybir.dt.float8e4)

    def prep_w(w, j_major=False):
        ws = np.where(np.asarray(w) >= 0, np.float32(1.0), np.float32(-1.0))
        # [3,3,ci,co] -> [p, (j,) tap, ktile, m]; ci = ktile*128+p, co = j*128+m
        wr = ws.reshape(9, 2, KP, NCHUNK, KP)
        wr = wr.transpose((2, 3, 0, 1, 4) if j_major else (2, 0, 3, 1, 4))
        return np.ascontiguousarray(wr).astype(fp8np)

    w1p, w2p = prep_w(w1, j_major=True), prep_w(w2)

    cpu = jax.devices("cpu")[0]
    MAXH = 9 * C
    with jax.default_device(cpu):
        hs = jnp.arange(-MAXH, MAXH + 1, dtype=jnp.float32)
        bn1 = (hs[:, None] - jnp.asarray(mean1)[None, :]) * lax.rsqrt(
            jnp.asarray(var1) + 1e-3
        )[None, :] + jnp.asarray(beta1)[None, :]
        nonneg = np.asarray(bn1 >= 0)
        r2 = np.asarray(lax.rsqrt(jnp.asarray(var2) + 1e-3))

    assert (np.diff(nonneg.astype(np.int8), axis=0) >= 0).all(), "bn1 not monotone"
    kc = np.where(nonneg.any(0), nonneg.argmax(0), 2 * MAXH + 1) - MAXH
    # device psum holds h/2 (x=+-0.5, w=+-1): sign flips at (kc-0.5)/2
    nt1 = (-(kc.astype(np.float64) - 0.5) / 2.0).astype(np.float32)

    s2 = r2.astype(np.float32)
    b2 = (
        np.asarray(beta2, np.float64)
        - np.asarray(mean2, np.float64) * s2.astype(np.float64)
    ).astype(np.float32)

    def to_pj(a):  # [256] -> [128, 2] with c = j*128+p
        return np.ascontiguousarray(a.reshape(NCHUNK, KP).T).astype(np.float32)

    # pack everything into one [128, CONST_B] uint8 image
    cbuf = np.zeros((KP, CONST_B), dtype=np.uint8)

    def put(off, arr):
        by = np.ascontiguousarray(arr).reshape(KP, -1).view(np.uint8)
        cbuf[:, off : off + by.shape[1]] = by

    put(W1J0_OFF, w1p[:, 0])
    put(W1J1_OFF, w1p[:, 1])
    put(W2_OFF, w2p)
    put(NT1_OFF, to_pj(nt1))
    put(S2_OFF, to_pj(s2))
    put(B2_OFF, to_pj(b2))
    return {"cb": cbuf}


# ---------------------------------------------------------------------------
# entry point
# ---------------------------------------------------------------------------

_cached = {}


def _run(inputs, trace=False):
    from concourse import bass_utils

    x = np.asarray(inputs["x"], dtype=np.float32)
    Bt, H, W, _ = x.shape  # 32, 56, 56, 256
    Bc = Bt // N_CORES

    consts = _prep_consts(
        inputs["w1"], inputs["beta1"], inputs["mean1"], inputs["var1"],
        inputs["w2"], inputs["beta2"], inputs["mean2"], inputs["var2"],
    )

    key = (Bc, H, W)
    if key not in _cached:
        _cached[key] = build_program(Bc, H, W)
    nc = _cached[key]

    # channel-major marshaling: [Bt, H*W, C] -> per-core [Bc, C, H*W]
    xcm = np.ascontiguousarray(
        x.reshape(Bt, H * W, C).transpose(0, 2, 1)
    )

    in_maps = []
    for c in range(N_CORES):
        m = dict(consts)
        m["x"] = xcm[c * Bc : (c + 1) * Bc]
        in_maps.append(m)

    res = bass_utils.run_bass_kernel_spmd(
        nc, in_maps, core_ids=list(range(N_CORES)), trace=trace
    )
    PO = (H // 2) * (W // 2)
    # y comes back channel-major [Bc, 2, 128, PO]; restore NHWC
    y = np.concatenate(
        [r["y"].reshape(Bc, C, PO).transpose(0, 2, 1) for r in res.results], axis=0
    )
    y = np.ascontiguousarray(y.reshape(Bt, H // 2, W // 2, C)).astype(np.float32)
    return y, res


def kernel(**inputs):
    y, _ = _run(inputs, trace=False)
    return y
